# revision 23
# baseline (speedup 1.0000x reference)
"""Trainium2 Bass kernel for nn_AgnosticResidualInteractionBlock (GNN message passing).

Strategy (8 NeuronCores, receiver-node graph partition):
  - Host: sort edges by receiver, shard receivers 2048/core, pad each
    128-receiver tile's edge segment to whole 128-edge chunks (uniform chunk
    counts across cores), species-permute each core's node shard, fold all
    constant scales into weights, and pre-layout every per-edge quantity in
    an edge-on-partition friendly layout.
  - Device (SPMD, one Bass program on 8 cores):
      A) per-species linear (sc) + pre-message linear (F) for the local node
         shard via TensorE (species-grouped tiles)
      B) AllGather of the bf16 F table (all cores need all sender features)
      C) radial MLP on edges (TensorE) + per-edge scalar table Q (DVE)
      D) main loop per node tile: dma_gather sender rows of F, build scaled
         one-hot scatter matrices (DVE/ACT), scatter-accumulate into PSUM via
         TensorE matmuls, then the post-message linear + output interleave.
  - Host: concatenate/unpermute shard outputs.
"""
import sys
import numpy as np

try:
    import concourse.bacc as bacc
except ImportError:  # pragma: no cover
    sys.path.insert(0, "/opt/trn_rl_repo")
    import concourse.bacc as bacc

import ml_dtypes
from contextlib import ExitStack

import concourse.bass as bass
import concourse.mybir as mybir
import concourse.tile as tile
from concourse import library_config
from concourse.bass_utils import run_bass_kernel_spmd

BF16 = ml_dtypes.bfloat16
N, C, E, S = 16384, 128, 262144, 10
NCORES = 8
NSH = N // NCORES            # 2048 nodes per core
NT = NSH // 128              # 16 node tiles per core
AVG = 16.0
INV_C = 1.0 / np.sqrt(C)
INV2C = 1.0 / np.sqrt(2 * C)
INV_SQRT3 = 1.0 / np.sqrt(3.0)

_CACHE = {}


def _host_prep(node_specie, node_feats, edge_attrs, edge_feats, senders, receivers,
               W_sc0, W_sc1, W_pre0, W_pre1, W_mlp1, W_mlp2, W_mlp3, W_post0, W_post1):
    """All numpy. Returns (meta, per_core_inputs, shared_inputs, unshard_info)."""
    senders = np.asarray(senders).astype(np.int64)
    receivers = np.asarray(receivers).astype(np.int64)
    node_specie = np.asarray(node_specie).astype(np.int64)
    node_feats = np.asarray(node_feats, dtype=np.float32)
    edge_attrs = np.asarray(edge_attrs, dtype=np.float32)
    edge_feats = np.asarray(edge_feats, dtype=np.float32)

    # ---- species permutation per core, uniform tile->species map -----------
    spec_counts = np.zeros((NCORES, S), np.int64)
    core_of_node = np.arange(N) // NSH
    for k in range(NCORES):
        sp = node_specie[k * NSH:(k + 1) * NSH]
        spec_counts[k] = np.bincount(sp, minlength=S)
    tiles_per_spec = np.maximum(1, -(-spec_counts.max(axis=0) // 128))  # ceil
    NSLOT = int(tiles_per_spec.sum() * 128)
    spec_tile_off = np.concatenate([[0], np.cumsum(tiles_per_spec)])[:-1] * 128
    species_of_tile = np.repeat(np.arange(S), tiles_per_spec)

    # slot_of[k, local_node] ; node_of[k, slot] (-1 = dummy)
    slot_of = np.zeros((NCORES, NSH), np.int64)
    node_of = -np.ones((NCORES, NSLOT), np.int64)
    for k in range(NCORES):
        sp = node_specie[k * NSH:(k + 1) * NSH]
        for s in range(S):
            loc = np.nonzero(sp == s)[0]
            slots = spec_tile_off[s] + np.arange(len(loc))
            slot_of[k, loc] = slots
            node_of[k, slots] = loc  # local node id
    assert N < 32768

    # ---- edge sort by receiver, tile segmentation, padding -----------------
    order = np.argsort(receivers, kind="stable")
    s_sorted = senders[order]
    r_sorted = receivers[order]
    a_sorted = edge_attrs[order]
    ef_sorted = edge_feats[order]
    # counts per (core, tile-index)
    gtile = r_sorted // 128  # 0..127
    cnt = np.bincount(gtile, minlength=128).reshape(NCORES, NT)
    chunks_t = np.maximum(1, -(-cnt.max(axis=0) // 128))  # [NT]
    NCH = int(chunks_t.sum())
    NCHP = -(-NCH // 32) * 32
    EPAD = NCH * 128
    EPADP = NCHP * 128
    off_t = np.concatenate([[0], np.cumsum(chunks_t)])[:-1]  # chunk offsets

    # per-core padded edge stream arrays
    eidx = np.zeros((NCORES, EPADP), np.int64)      # F row per edge
    rloc = -np.ones((NCORES, EPADP), np.float32)    # local receiver 0..127
    aarr = np.zeros((NCORES, EPADP, 3), np.float32)
    efarr = np.zeros((NCORES, EPADP, 8), np.float32)
    gstart = np.concatenate([[0], np.cumsum(cnt.reshape(-1))])
    for k in range(NCORES):
        for t in range(NT):
            g = k * NT + t
            seg = slice(gstart[g], gstart[g + 1])
            n = gstart[g + 1] - gstart[g]
            base = int(off_t[t]) * 128
            eidx[k, base:base + n] = s_sorted[seg]
            rloc[k, base:base + n] = (r_sorted[seg] % 128).astype(np.float32)
            aarr[k, base:base + n] = a_sorted[seg, 1:4]
            efarr[k, base:base + n] = ef_sorted[seg]

    # ---- weights with folded scales ---------------------------------------
    wsc0T = (np.asarray(W_sc0, np.float32) * INV_C).transpose(1, 0, 2).reshape(128, S * 128)
    wsc1T = (np.asarray(W_sc1, np.float32) * INV_C).transpose(1, 0, 2).reshape(128, S * 128)
    wpre = np.concatenate([np.asarray(W_pre0, np.float32) * INV_C,
                           np.asarray(W_pre1, np.float32) * INV_C], axis=1)  # [128, 256]
    wm1 = (np.asarray(W_mlp1, np.float32) / np.sqrt(8.0)).astype(BF16)
    wm2 = (np.asarray(W_mlp2, np.float32) * 0.125).astype(BF16)
    wm3 = (np.asarray(W_mlp3, np.float32) * 0.125).astype(BF16)
    sc_post = INV2C / AVG
    wp = np.concatenate([
        np.asarray(W_post0, np.float32)[:, 0, :] * sc_post,
        np.asarray(W_post0, np.float32)[:, 1, :] * sc_post * INV_SQRT3,
        np.asarray(W_post1, np.float32)[:, 0, :] * sc_post,
        np.asarray(W_post1, np.float32)[:, 1, :] * sc_post,
    ], axis=1).astype(BF16)  # [128, 512]

    # ---- per-core device arrays -------------------------------------------
    per_core = []
    for k in range(NCORES):
        nf = node_feats[k * NSH:(k + 1) * NSH]  # [2048, 128, 4]
        nfT = np.zeros((4, 128, NSLOT), np.float32)
        valid = node_of[k] >= 0
        nfT[:, :, valid] = nf[node_of[k][valid]].transpose(2, 1, 0)
        # idx wrap for dma_gather: idx i -> partition i%16, col i//16, tile 8x
        iw = eidx[k].reshape(EPADP // 16, 16).T.astype(np.int16)
        iwr = np.tile(iw, (8, 1))
        rlocT = np.ascontiguousarray(
            rloc[k].reshape(NCHP, 128).T).astype(np.float32)  # [128, NCHP]
        ohT = (rloc[k].reshape(NCHP, 128, 1) ==
               np.arange(128, dtype=np.float32)).astype(BF16)
        ohT = np.ascontiguousarray(ohT.transpose(1, 0, 2).reshape(128, NCHP * 128))
        aT = np.ascontiguousarray(
            aarr[k].reshape(NCHP, 128, 3).transpose(1, 0, 2).reshape(128, NCHP * 3))
        efT = np.ascontiguousarray(efarr[k].reshape(EPADP, 8).T).astype(BF16)
        per_core.append(dict(
            nfT=nfT, eidx=iwr, rlocT=rlocT, aT=aT.astype(np.float32), efT=efT,
            ohT=ohT,
        ))

    iota = np.tile(np.arange(128, dtype=np.float32), (128, 1)).astype(BF16)
    nfTb = np.ascontiguousarray(node_feats.transpose(2, 1, 0)).astype(BF16)
    shared = dict(wsc0T=wsc0T, wsc1T=wsc1T, wpre=wpre, wpre_b=wpre.astype(BF16),
                  nfTb=nfTb, wm1=wm1, wm2=wm2, wm3=wm3, wp=wp, iota=iota)
    meta = dict(NSLOT=NSLOT, NCH=NCH, NCHP=NCHP, EPADP=EPADP,
                chunks_t=tuple(int(x) for x in chunks_t),
                off_t=tuple(int(x) for x in off_t),
                species_of_tile=tuple(int(x) for x in species_of_tile))
    unshard = dict(node_of=node_of, NSLOT=NSLOT)
    return meta, per_core, shared, unshard


def _dep(later, earlier):
    tile.add_dep_helper(later.ins, earlier.ins, sync=False, reason="psum group order")


def _build(meta):
    NSLOT = meta["NSLOT"]
    NCH, NCHP, EPADP = meta["NCH"], meta["NCHP"], meta["EPADP"]
    chunks_t, off_t = meta["chunks_t"], meta["off_t"]
    species_of_tile = meta["species_of_tile"]
    NGT = NSLOT // 128
    MAXCH = max(chunks_t)
    f32, bf16, i16 = mybir.dt.float32, mybir.dt.bfloat16, mybir.dt.int16

    nc = bacc.Bacc("TRN2", target_bir_lowering=False)
    # inputs
    nfT = nc.dram_tensor("nfT", [4, 128, NSLOT], f32, kind="ExternalInput")
    eidx = nc.dram_tensor("eidx", [128, EPADP // 16], i16, kind="ExternalInput")
    rlocT = nc.dram_tensor("rlocT", [128, NCHP], f32, kind="ExternalInput")
    aT = nc.dram_tensor("aT", [128, NCHP * 3], f32, kind="ExternalInput")
    ohT = nc.dram_tensor("ohT", [128, NCHP * 128], bf16, kind="ExternalInput")
    efT = nc.dram_tensor("efT", [8, EPADP], bf16, kind="ExternalInput")
    wsc0T = nc.dram_tensor("wsc0T", [128, S * 128], f32, kind="ExternalInput")
    wsc1T = nc.dram_tensor("wsc1T", [128, S * 128], f32, kind="ExternalInput")
    wpre = nc.dram_tensor("wpre", [128, 256], f32, kind="ExternalInput")
    wpre_b = nc.dram_tensor("wpre_b", [128, 256], bf16, kind="ExternalInput")
    nfTb = nc.dram_tensor("nfTb", [4, 128, N], bf16, kind="ExternalInput")
    wm1 = nc.dram_tensor("wm1", [8, 64], bf16, kind="ExternalInput")
    wm2 = nc.dram_tensor("wm2", [64, 64], bf16, kind="ExternalInput")
    wm3 = nc.dram_tensor("wm3", [64, 4], bf16, kind="ExternalInput")
    wp = nc.dram_tensor("wp", [128, 512], bf16, kind="ExternalInput")
    iota = nc.dram_tensor("iota", [128, 128], bf16, kind="ExternalInput")
    # outputs
    sc_out = nc.dram_tensor("sc_out", [NSLOT, 512], f32, kind="ExternalOutput")
    nout = nc.dram_tensor("nout", [NSH, 512], f32, kind="ExternalOutput")

    with tile.TileContext(nc) as tc, ExitStack() as ctx:
        nc.gpsimd.load_library(library_config.mlp)
        dr = ctx.enter_context(tc.tile_pool(name="dr", bufs=1, space="DRAM"))
        F_full = dr.tile([N, 512], bf16)

        cst = ctx.enter_context(tc.tile_pool(name="cst", bufs=1))
        iota_sb = cst.tile([128, 128], bf16)
        nc.sync.dma_start(iota_sb[:], iota[:])
        wp_sb = cst.tile([128, 512], bf16)
        nc.sync.dma_start(wp_sb[:], wp[:])
        wm1_sb = cst.tile([8, 64], bf16)
        nc.sync.dma_start(wm1_sb[:], wm1[:])
        wm2_sb = cst.tile([64, 64], bf16)
        nc.sync.dma_start(wm2_sb[:], wm2[:])
        wm3_sb = cst.tile([64, 4], bf16)
        nc.sync.dma_start(wm3_sb[:], wm3[:])
        eidx_sb = cst.tile([128, EPADP // 16], i16)
        nc.sync.dma_start(eidx_sb[:], eidx[:])
        rloc_sb = cst.tile([128, NCHP], f32)
        nc.sync.dma_start(rloc_sb[:], rlocT[:])
        aT_sb = cst.tile([128, NCHP * 3], f32)
        nc.sync.dma_start(aT_sb[:], aT[:])
        q_sb = cst.tile([128, NCHP * 8], f32)

        # ---- stages A (sc), C (MLP+Q), then premp-all; one PSUM scope ------
        with tc.tile_pool(name="sa", bufs=1) as sa, \
             tc.tile_pool(name="sa2", bufs=3) as sa2, \
             tc.tile_pool(name="mc", bufs=3) as mc, \
             tc.tile_pool(name="psA", bufs=2, space="PSUM") as psA:
            wsc0_sb = sa.tile([128, S * 128], f32)
            nc.sync.dma_start(wsc0_sb[:], wsc0T[:])
            wsc1_sb = sa.tile([128, S * 128], f32)
            nc.sync.dma_start(wsc1_sb[:], wsc1T[:])
            ef_sb = sa.tile([8, EPADP], bf16)
            nc.sync.dma_start(ef_sb[:], efT[:])
            wpreb_sb = sa.tile([128, 256], bf16)
            nc.sync.dma_start(wpreb_sb[:], wpre_b[:])
            nfc = []
            for comp in range(4):
                t = sa.tile([128, NSLOT], f32, name=f"nfc{comp}")
                nc.sync.dma_start(t[:], nfT[comp, :, :])
                nfc.append(t)
            # ---- A: per-species sc for the local shard ----
            for gt in range(NGT):
                sp = species_of_tile[gt]
                ps_sc = psA.tile([128, 512], f32, tag="ps_sc", space="PSUM")
                mm_chain = []
                for comp in range(4):
                    lhsT = nfc[comp][:, gt * 128:(gt + 1) * 128]
                    wsc = (wsc0_sb if comp == 0 else wsc1_sb)[:, sp * 128:(sp + 1) * 128]
                    mm_chain.append(nc.tensor.matmul(
                        ps_sc[:, comp * 128:(comp + 1) * 128],
                        lhsT=lhsT, rhs=wsc, start=True, stop=True))
                for a, b in zip(mm_chain, mm_chain[1:]):
                    _dep(b, a)
                sc_sb = sa2.tile([128, 512], f32, tag="sc_sb")
                nc.vector.tensor_copy(sc_sb[:], ps_sc[:])
                nc.sync.dma_start(sc_out[gt * 128:(gt + 1) * 128, :], sc_sb[:])
            # ---- C: radial MLP + per-edge scalar table Q ----
            for gm in range(NCHP // 32):   # 32 chunks = 4096 edges per group
                ps_mix = psA.tile([128, 128], f32, tag="ps_mix", space="PSUM")
                mix_chain = []
                for g5 in range(8):        # 512-edge subgroups
                    e0 = gm * 4096 + g5 * 512
                    ps_h = psA.tile([64, 512], f32, tag="ps_h", space="PSUM")
                    nc.tensor.matmul(ps_h[:], lhsT=wm1_sb[:], rhs=ef_sb[:, e0:e0 + 512],
                                     start=True, stop=True)
                    h1 = mc.tile([64, 512], bf16, tag="h1")
                    nc.scalar.activation(h1[:], ps_h[:], mybir.ActivationFunctionType.Silu)
                    ps_h2 = psA.tile([64, 512], f32, tag="ps_h2", space="PSUM")
                    nc.tensor.matmul(ps_h2[:], lhsT=wm2_sb[:], rhs=h1[:],
                                     start=True, stop=True)
                    h2 = mc.tile([64, 512], bf16, tag="h2")
                    nc.scalar.activation(h2[:], ps_h2[:], mybir.ActivationFunctionType.Silu)
                    for j in range(4):
                        cc = g5 * 4 + j
                        mix_chain.append(nc.tensor.matmul(
                            ps_mix[:, cc * 4:cc * 4 + 4],
                            lhsT=h2[:, j * 128:(j + 1) * 128], rhs=wm3_sb[:],
                            start=True, stop=True))
                for a, b in zip(mix_chain, mix_chain[1:]):
                    _dep(b, a)
                mix_sb = mc.tile([128, 128], f32, tag="mix_sb")
                nc.vector.tensor_copy(mix_sb[:], ps_mix[:])
                # Q build for these 32 chunks
                qs = q_sb[:, gm * 256:(gm + 1) * 256]
                mix4 = mix_sb[:].rearrange("p (c f) -> p c f", f=4)
                q8 = qs.rearrange("p (c f) -> p c f", f=8)
                a3 = aT_sb[:, gm * 96:(gm + 1) * 96].rearrange("p (c f) -> p c f", f=3)
                nc.vector.tensor_copy(q8[:, :, 0:1], mix4[:, :, 0:1])
                nc.vector.tensor_tensor(out=q8[:, :, 1:4], in0=a3[:, :, :],
                                        in1=mix4[:, :, 1:2].to_broadcast([128, 32, 3]),
                                        op=mybir.AluOpType.mult)
                nc.vector.tensor_copy(q8[:, :, 4:5], mix4[:, :, 2:3])
                nc.vector.tensor_tensor(out=q8[:, :, 5:8], in0=a3[:, :, :],
                                        in1=mix4[:, :, 3:4].to_broadcast([128, 32, 3]),
                                        op=mybir.AluOpType.mult)
            # ---- premp for ALL cores' nodes (replaces the AllGather) ------
            NR = N
            for g8 in range(NR // 1024):
                nfb = []
                for comp in range(4):
                    tb = sa2.tile([128, 1024], bf16, tag=f"nfb{comp}", name=f"nfb{comp}")
                    nc.sync.dma_start(tb[:], nfTb[comp, :, g8 * 1024:(g8 + 1) * 1024])
                    nfb.append(tb)
                f_wide = sa2.tile([128, 8, 512], bf16, tag="f_wide")
                for sub in range(8):
                    ps_f = psA.tile([128, 512], f32, tag="ps_sc", space="PSUM")
                    fchain = []
                    for comp in range(4):
                        wpr = wpreb_sb[:, 0:128] if comp == 0 else wpreb_sb[:, 128:256]
                        fchain.append(nc.tensor.matmul(
                            ps_f[:, comp * 128:(comp + 1) * 128],
                            lhsT=nfb[comp][:, sub * 128:(sub + 1) * 128],
                            rhs=wpr, start=True, stop=True))
                    for a, b in zip(fchain, fchain[1:]):
                        _dep(b, a)
                    nc.vector.tensor_copy(f_wide[:, sub, :], ps_f[:])
                nc.sync.dma_start(
                    F_full[g8 * 1024:(g8 + 1) * 1024, :].rearrange(
                        "(s p) c -> p s c", p=128),
                    f_wide[:])

        # ---------------- stage D: gather / scatter / postmp ----------------
        with tc.tile_pool(name="sd", bufs=2) as sd, \
             tc.tile_pool(name="sdh", bufs=3) as sdh, \
             tc.tile_pool(name="psagg", bufs=2, space="PSUM") as psagg, \
             tc.tile_pool(name="pso", bufs=2, space="PSUM") as pso:
            for t in range(NT):
                nch = chunks_t[t]
                c0 = off_t[t]
                fg = sd.tile([128, MAXCH, 512], bf16, tag="fg")
                oh_sl = sd.tile([128, MAXCH, 128], bf16, tag="oh_sl")
                nc.sync.dma_start(oh_sl[:, :nch, :],
                                  ohT[:, c0 * 128:(c0 + nch) * 128].rearrange(
                                      "p (j n) -> p j n", n=128))
                # dma_gather ucode handles at most 1024 indices per call
                for b0 in range(0, nch, 8):
                    bn = min(8, nch - b0)
                    nc.gpsimd.dma_gather(
                        fg[:, b0:b0 + bn, :], F_full[:],
                        eidx_sb[:, (c0 + b0) * 8:(c0 + b0 + bn) * 8],
                        bn * 128, bn * 128, 512)
                agg = psagg.tile([128, 1024], f32, tag="agg", space="PSUM")
                # PSUM accumulation groups are per 2KB bank: exactly one MM
                # per bank carries start=True (pending-zeroes the whole bank),
                # one carries stop=True, and explicit deps order them.
                bank_mms = [[], []]  # instructions per psum bank
                for j in range(nch):
                    ch = c0 + j
                    rl = rloc_sb[:, ch:ch + 1]
                    q = [q_sb[:, ch * 8 + i:ch * 8 + i + 1] for i in range(8)]
                    h = [sdh.tile([128, 128], bf16, tag=f"h{i}", name=f"h{i}")
                         for i in range(8)]
                    # DVE: 6 fused (iota==rloc)*q at 4x mode; ACT: 2 scaled
                    # copies of the host-precomputed one-hot slab
                    for i in range(6):
                        nc.vector.tensor_scalar(
                            h[i][:], iota_sb[:], rl, q[i][:],
                            mybir.AluOpType.is_equal, mybir.AluOpType.mult)
                    for i in range(6, 8):
                        nc.scalar.mul(h[i][:], oh_sl[:, j, :], q[i][:])
                    h = [t_[:] for t_ in h]
                    fs = fg[:, j, 0:128]
                    fvx = fg[:, j, 128:256]
                    fvy = fg[:, j, 256:384]
                    fvz = fg[:, j, 384:512]
                    # (column block, lhsT, rhs): block 0: s*mix0; 1: dot;
                    # 2-4: v_i*mix2; 5-7: s*a_i*mix3
                    for col, lhsT, rhs in (
                        (0, fs, h[0]), (640, fs, h[5]), (768, fs, h[6]),
                        (896, fs, h[7]), (128, fvx, h[1]), (256, fvx, h[4]),
                        (128, fvy, h[2]), (384, fvy, h[4]), (128, fvz, h[3]),
                        (512, fvz, h[4]),
                    ):
                        bank = col // 512
                        first = j == 0 and not bank_mms[bank]
                        inst = nc.tensor.matmul(
                            agg[:, col:col + 128], lhsT=lhsT, rhs=rhs,
                            start=first, stop=False)
                        bank_mms[bank].append(inst)
                for bank in range(2):
                    mms = bank_mms[bank]
                    mms[-1].ins.stop_tensor_calc = True
                    for m in mms[1:]:
                        _dep(m, mms[0])
                    for m in mms[:-1]:
                        _dep(mms[-1], m)
                agg_sb = sd.tile([128, 1024], bf16, tag="agg_sb")
                nc.scalar.copy(agg_sb[:], agg[:])
                o_ps = pso.tile([128, 512], f32, tag="o_ps", space="PSUM")
                # os = aggT0 @ Wp00 + aggT1 @ Wp01 ; ov_i = aggT(2+i) @ Wp10 + aggT(5+i) @ Wp11
                o_chain = [nc.tensor.matmul(
                    o_ps[:, 0:128], lhsT=agg_sb[:, 0:128], rhs=wp_sb[:, 0:128],
                    start=True, stop=False)]
                o_chain.append(nc.tensor.matmul(
                    o_ps[:, 0:128], lhsT=agg_sb[:, 128:256], rhs=wp_sb[:, 128:256],
                    start=False, stop=True))
                for i in range(3):
                    o_chain.append(nc.tensor.matmul(
                        o_ps[:, 128 * (i + 1):128 * (i + 2)],
                        lhsT=agg_sb[:, 128 * (2 + i):128 * (3 + i)],
                        rhs=wp_sb[:, 256:384], start=True, stop=False))
                    o_chain.append(nc.tensor.matmul(
                        o_ps[:, 128 * (i + 1):128 * (i + 2)],
                        lhsT=agg_sb[:, 128 * (5 + i):128 * (6 + i)],
                        rhs=wp_sb[:, 384:512], start=False, stop=True))
                for a, b in zip(o_chain, o_chain[1:]):
                    _dep(b, a)
                out_sb = sd.tile([128, 512], f32, tag="out_sb")
                nc.scalar.copy(
                    out_sb[:].rearrange("p (d c) -> p c d", c=4),
                    o_ps[:].rearrange("p (c d) -> p c d", c=4))
                nc.sync.dma_start(nout[t * 128:(t + 1) * 128, :], out_sb[:])

    nc.compile()
    return nc


def kernel(**inputs):
    meta, per_core, shared, unshard = _host_prep(**inputs)
    key = (meta["NSLOT"], meta["NCH"], meta["chunks_t"], meta["species_of_tile"])
    if key not in _CACHE:
        _CACHE[key] = _build(meta)
    nc = _CACHE[key]
    in_maps = [dict(pc, **shared) for pc in per_core]
    res = run_bass_kernel_spmd(nc, in_maps, core_ids=list(range(NCORES)))
    node_out = np.concatenate([res.results[k]["nout"] for k in range(NCORES)], axis=0)
    node_out = node_out.reshape(N, 128, 4)
    node_of = unshard["node_of"]
    sc = np.zeros((N, 128, 4), np.float32)
    for k in range(NCORES):
        valid = node_of[k] >= 0
        rows = res.results[k]["sc_out"][valid]
        sc[k * NSH + node_of[k][valid]] = rows.reshape(-1, 4, 128).transpose(0, 2, 1)
    return node_out, sc


# revision 24
# speedup vs baseline: 23429.5979x; 23429.5979x over previous
"""Trainium2 Bass kernel for nn_AgnosticResidualInteractionBlock (GNN message passing).

Strategy (8 NeuronCores, receiver-node graph partition):
  - Host: sort edges by receiver, shard receivers 2048/core, pad each
    128-receiver tile's edge segment to whole 128-edge chunks (uniform chunk
    counts across cores), species-permute each core's node shard, fold all
    constant scales into weights, and pre-layout every per-edge quantity in
    an edge-on-partition friendly layout.
  - Device (SPMD, one Bass program on 8 cores):
      A) per-species linear (sc) for the local shard (species-grouped tiles),
         radial MLP on edges + per-edge scalar table Q, and the pre-message
         linear F for ALL nodes in bf16 (replicated compute instead of an
         AllGather: each core owns a full local F table) -- all overlapped
         across TensorE / ACT / DVE / three DMA queues.
      D) main loop per node tile: dma_gather sender rows of F (<=1024 idx per
         call, a ucode limit), build scaled one-hot scatter matrices (fused
         is_equal*q tensor_scalar on DVE at 4x + scaled copies of a
         host-precomputed one-hot slab on ACT), scatter-accumulate aggT in
         PSUM via TensorE (one accumulation group per 2KB PSUM bank, ordered
         by explicit deps), then the post-message linear and the interleaved
         output eviction.
  - Host: concatenate/unpermute shard outputs.
"""
import sys
import numpy as np

try:
    import concourse.bacc as bacc
except ImportError:  # pragma: no cover
    sys.path.insert(0, "/opt/trn_rl_repo")
    import concourse.bacc as bacc

import ml_dtypes
from contextlib import ExitStack

import concourse.bass as bass
import concourse.mybir as mybir
import concourse.tile as tile
from concourse import library_config
from concourse.bass_utils import run_bass_kernel_spmd

BF16 = ml_dtypes.bfloat16
N, C, E, S = 16384, 128, 262144, 10
NCORES = 8
NSH = N // NCORES            # 2048 nodes per core
NT = NSH // 128              # 16 node tiles per core
AVG = 16.0
INV_C = 1.0 / np.sqrt(C)
INV2C = 1.0 / np.sqrt(2 * C)
INV_SQRT3 = 1.0 / np.sqrt(3.0)

_CACHE = {}


def _host_prep(node_specie, node_feats, edge_attrs, edge_feats, senders, receivers,
               W_sc0, W_sc1, W_pre0, W_pre1, W_mlp1, W_mlp2, W_mlp3, W_post0, W_post1):
    """All numpy. Returns (meta, per_core_inputs, shared_inputs, unshard_info)."""
    senders = np.asarray(senders).astype(np.int64)
    receivers = np.asarray(receivers).astype(np.int64)
    node_specie = np.asarray(node_specie).astype(np.int64)
    node_feats = np.asarray(node_feats, dtype=np.float32)
    edge_attrs = np.asarray(edge_attrs, dtype=np.float32)
    edge_feats = np.asarray(edge_feats, dtype=np.float32)

    # ---- species permutation per core, uniform tile->species map -----------
    spec_counts = np.zeros((NCORES, S), np.int64)
    core_of_node = np.arange(N) // NSH
    for k in range(NCORES):
        sp = node_specie[k * NSH:(k + 1) * NSH]
        spec_counts[k] = np.bincount(sp, minlength=S)
    tiles_per_spec = np.maximum(1, -(-spec_counts.max(axis=0) // 128))  # ceil
    NSLOT = int(tiles_per_spec.sum() * 128)
    spec_tile_off = np.concatenate([[0], np.cumsum(tiles_per_spec)])[:-1] * 128
    species_of_tile = np.repeat(np.arange(S), tiles_per_spec)

    # slot_of[k, local_node] ; node_of[k, slot] (-1 = dummy)
    slot_of = np.zeros((NCORES, NSH), np.int64)
    node_of = -np.ones((NCORES, NSLOT), np.int64)
    for k in range(NCORES):
        sp = node_specie[k * NSH:(k + 1) * NSH]
        for s in range(S):
            loc = np.nonzero(sp == s)[0]
            slots = spec_tile_off[s] + np.arange(len(loc))
            slot_of[k, loc] = slots
            node_of[k, slots] = loc  # local node id
    assert N < 32768

    # ---- edge sort by receiver, tile segmentation, padding -----------------
    order = np.argsort(receivers, kind="stable")
    s_sorted = senders[order]
    r_sorted = receivers[order]
    a_sorted = edge_attrs[order]
    ef_sorted = edge_feats[order]
    # counts per (core, tile-index)
    gtile = r_sorted // 128  # 0..127
    cnt = np.bincount(gtile, minlength=128).reshape(NCORES, NT)
    chunks_t = np.maximum(1, -(-cnt.max(axis=0) // 128))  # [NT]
    NCH = int(chunks_t.sum())
    NCHP = -(-NCH // 32) * 32
    EPAD = NCH * 128
    EPADP = NCHP * 128
    off_t = np.concatenate([[0], np.cumsum(chunks_t)])[:-1]  # chunk offsets

    # per-core padded edge stream arrays
    eidx = np.zeros((NCORES, EPADP), np.int64)      # F row per edge
    rloc = -np.ones((NCORES, EPADP), np.float32)    # local receiver 0..127
    aarr = np.zeros((NCORES, EPADP, 3), np.float32)
    efarr = np.zeros((NCORES, EPADP, 8), np.float32)
    gstart = np.concatenate([[0], np.cumsum(cnt.reshape(-1))])
    for k in range(NCORES):
        for t in range(NT):
            g = k * NT + t
            seg = slice(gstart[g], gstart[g + 1])
            n = gstart[g + 1] - gstart[g]
            base = int(off_t[t]) * 128
            eidx[k, base:base + n] = s_sorted[seg]
            rloc[k, base:base + n] = (r_sorted[seg] % 128).astype(np.float32)
            aarr[k, base:base + n] = a_sorted[seg, 1:4]
            efarr[k, base:base + n] = ef_sorted[seg]

    # ---- weights with folded scales ---------------------------------------
    wsc0T = (np.asarray(W_sc0, np.float32) * INV_C).transpose(1, 0, 2).reshape(128, S * 128)
    wsc1T = (np.asarray(W_sc1, np.float32) * INV_C).transpose(1, 0, 2).reshape(128, S * 128)
    wpre = np.concatenate([np.asarray(W_pre0, np.float32) * INV_C,
                           np.asarray(W_pre1, np.float32) * INV_C], axis=1)  # [128, 256]
    wm1 = (np.asarray(W_mlp1, np.float32) / np.sqrt(8.0)).astype(BF16)
    wm2 = (np.asarray(W_mlp2, np.float32) * 0.125).astype(BF16)
    wm3 = (np.asarray(W_mlp3, np.float32) * 0.125).astype(BF16)
    sc_post = INV2C / AVG
    wp = np.concatenate([
        np.asarray(W_post0, np.float32)[:, 0, :] * sc_post,
        np.asarray(W_post0, np.float32)[:, 1, :] * sc_post * INV_SQRT3,
        np.asarray(W_post1, np.float32)[:, 0, :] * sc_post,
        np.asarray(W_post1, np.float32)[:, 1, :] * sc_post,
    ], axis=1).astype(BF16)  # [128, 512]

    # ---- per-core device arrays -------------------------------------------
    per_core = []
    for k in range(NCORES):
        nf = node_feats[k * NSH:(k + 1) * NSH]  # [2048, 128, 4]
        nfT = np.zeros((4, 128, NSLOT), np.float32)
        valid = node_of[k] >= 0
        nfT[:, :, valid] = nf[node_of[k][valid]].transpose(2, 1, 0)
        # idx wrap for dma_gather: idx i -> partition i%16, col i//16, tile 8x
        iw = eidx[k].reshape(EPADP // 16, 16).T.astype(np.int16)
        iwr = np.tile(iw, (8, 1))
        rlocT = np.ascontiguousarray(
            rloc[k].reshape(NCHP, 128).T).astype(np.float32)  # [128, NCHP]
        ohT = (rloc[k].reshape(NCHP, 128, 1) ==
               np.arange(128, dtype=np.float32)).astype(BF16)
        ohT = np.ascontiguousarray(ohT.transpose(1, 0, 2).reshape(128, NCHP * 128))
        aT = np.ascontiguousarray(
            aarr[k].reshape(NCHP, 128, 3).transpose(1, 0, 2).reshape(128, NCHP * 3))
        efT = np.ascontiguousarray(efarr[k].reshape(EPADP, 8).T).astype(BF16)
        per_core.append(dict(
            nfT=nfT, eidx=iwr, rlocT=rlocT, aT=aT.astype(np.float32), efT=efT,
            ohT=ohT,
        ))

    iota = np.tile(np.arange(128, dtype=np.float32), (128, 1)).astype(BF16)
    nfTb = np.ascontiguousarray(node_feats.transpose(2, 1, 0)).astype(BF16)
    shared = dict(wsc0T=wsc0T, wsc1T=wsc1T, wpre=wpre, wpre_b=wpre.astype(BF16),
                  nfTb=nfTb, wm1=wm1, wm2=wm2, wm3=wm3, wp=wp, iota=iota)
    meta = dict(NSLOT=NSLOT, NCH=NCH, NCHP=NCHP, EPADP=EPADP,
                chunks_t=tuple(int(x) for x in chunks_t),
                off_t=tuple(int(x) for x in off_t),
                species_of_tile=tuple(int(x) for x in species_of_tile))
    unshard = dict(node_of=node_of, NSLOT=NSLOT)
    return meta, per_core, shared, unshard


def _dep(later, earlier):
    tile.add_dep_helper(later.ins, earlier.ins, sync=False, reason="psum group order")


def _build(meta):
    NSLOT = meta["NSLOT"]
    NCH, NCHP, EPADP = meta["NCH"], meta["NCHP"], meta["EPADP"]
    chunks_t, off_t = meta["chunks_t"], meta["off_t"]
    species_of_tile = meta["species_of_tile"]
    NGT = NSLOT // 128
    MAXCH = max(chunks_t)
    f32, bf16, i16 = mybir.dt.float32, mybir.dt.bfloat16, mybir.dt.int16

    nc = bacc.Bacc("TRN2", target_bir_lowering=False)
    # inputs
    nfT = nc.dram_tensor("nfT", [4, 128, NSLOT], f32, kind="ExternalInput")
    eidx = nc.dram_tensor("eidx", [128, EPADP // 16], i16, kind="ExternalInput")
    rlocT = nc.dram_tensor("rlocT", [128, NCHP], f32, kind="ExternalInput")
    aT = nc.dram_tensor("aT", [128, NCHP * 3], f32, kind="ExternalInput")
    ohT = nc.dram_tensor("ohT", [128, NCHP * 128], bf16, kind="ExternalInput")
    efT = nc.dram_tensor("efT", [8, EPADP], bf16, kind="ExternalInput")
    wsc0T = nc.dram_tensor("wsc0T", [128, S * 128], f32, kind="ExternalInput")
    wsc1T = nc.dram_tensor("wsc1T", [128, S * 128], f32, kind="ExternalInput")
    wpre = nc.dram_tensor("wpre", [128, 256], f32, kind="ExternalInput")
    wpre_b = nc.dram_tensor("wpre_b", [128, 256], bf16, kind="ExternalInput")
    nfTb = nc.dram_tensor("nfTb", [4, 128, N], bf16, kind="ExternalInput")
    wm1 = nc.dram_tensor("wm1", [8, 64], bf16, kind="ExternalInput")
    wm2 = nc.dram_tensor("wm2", [64, 64], bf16, kind="ExternalInput")
    wm3 = nc.dram_tensor("wm3", [64, 4], bf16, kind="ExternalInput")
    wp = nc.dram_tensor("wp", [128, 512], bf16, kind="ExternalInput")
    iota = nc.dram_tensor("iota", [128, 128], bf16, kind="ExternalInput")
    # outputs
    sc_out = nc.dram_tensor("sc_out", [NSLOT, 512], f32, kind="ExternalOutput")
    nout = nc.dram_tensor("nout", [NSH, 512], f32, kind="ExternalOutput")

    with tile.TileContext(nc) as tc, ExitStack() as ctx:
        nc.gpsimd.load_library(library_config.mlp)
        dr = ctx.enter_context(tc.tile_pool(name="dr", bufs=1, space="DRAM"))
        F_full = dr.tile([N, 512], bf16)

        cst = ctx.enter_context(tc.tile_pool(name="cst", bufs=1))
        iota_sb = cst.tile([128, 128], bf16)
        nc.sync.dma_start(iota_sb[:], iota[:])
        wp_sb = cst.tile([128, 512], bf16)
        nc.sync.dma_start(wp_sb[:], wp[:])
        wm1_sb = cst.tile([8, 64], bf16)
        nc.sync.dma_start(wm1_sb[:], wm1[:])
        wm2_sb = cst.tile([64, 64], bf16)
        nc.sync.dma_start(wm2_sb[:], wm2[:])
        wm3_sb = cst.tile([64, 4], bf16)
        nc.sync.dma_start(wm3_sb[:], wm3[:])
        eidx_sb = cst.tile([128, EPADP // 16], i16)
        nc.sync.dma_start(eidx_sb[:], eidx[:])
        rloc_sb = cst.tile([128, NCHP], f32)
        nc.sync.dma_start(rloc_sb[:], rlocT[:])
        aT_sb = cst.tile([128, NCHP * 3], f32)
        nc.sync.dma_start(aT_sb[:], aT[:])
        q_sb = cst.tile([128, NCHP * 8], f32)

        # ---- stages A (sc), C (MLP+Q), then premp-all; one PSUM scope ------
        with tc.tile_pool(name="sa", bufs=1) as sa, \
             tc.tile_pool(name="sa2", bufs=3) as sa2, \
             tc.tile_pool(name="mc", bufs=3) as mc, \
             tc.tile_pool(name="psA", bufs=2, space="PSUM") as psA:
            wsc0_sb = sa.tile([128, S * 128], f32)
            nc.sync.dma_start(wsc0_sb[:], wsc0T[:])
            wsc1_sb = sa.tile([128, S * 128], f32)
            nc.sync.dma_start(wsc1_sb[:], wsc1T[:])
            ef_sb = sa.tile([8, EPADP], bf16)
            nc.sync.dma_start(ef_sb[:], efT[:])
            wpreb_sb = sa.tile([128, 256], bf16)
            nc.sync.dma_start(wpreb_sb[:], wpre_b[:])
            nfc = []
            for comp in range(4):
                t = sa.tile([128, NSLOT], f32, name=f"nfc{comp}")
                nc.sync.dma_start(t[:], nfT[comp, :, :])
                nfc.append(t)
            # ---- A: per-species sc for the local shard ----
            for gt in range(NGT):
                sp = species_of_tile[gt]
                ps_sc = psA.tile([128, 512], f32, tag="ps_sc", space="PSUM")
                mm_chain = []
                for comp in range(4):
                    lhsT = nfc[comp][:, gt * 128:(gt + 1) * 128]
                    wsc = (wsc0_sb if comp == 0 else wsc1_sb)[:, sp * 128:(sp + 1) * 128]
                    mm_chain.append(nc.tensor.matmul(
                        ps_sc[:, comp * 128:(comp + 1) * 128],
                        lhsT=lhsT, rhs=wsc, start=True, stop=True))
                for a, b in zip(mm_chain, mm_chain[1:]):
                    _dep(b, a)
                sc_sb = sa2.tile([128, 512], f32, tag="sc_sb")
                nc.vector.tensor_copy(sc_sb[:], ps_sc[:])
                nc.sync.dma_start(sc_out[gt * 128:(gt + 1) * 128, :], sc_sb[:])
            # ---- C: radial MLP + per-edge scalar table Q ----
            for gm in range(NCHP // 32):   # 32 chunks = 4096 edges per group
                ps_mix = psA.tile([128, 128], f32, tag="ps_mix", space="PSUM")
                mix_chain = []
                for g5 in range(8):        # 512-edge subgroups
                    e0 = gm * 4096 + g5 * 512
                    ps_h = psA.tile([64, 512], f32, tag="ps_h", space="PSUM")
                    nc.tensor.matmul(ps_h[:], lhsT=wm1_sb[:], rhs=ef_sb[:, e0:e0 + 512],
                                     start=True, stop=True)
                    h1 = mc.tile([64, 512], bf16, tag="h1")
                    nc.scalar.activation(h1[:], ps_h[:], mybir.ActivationFunctionType.Silu)
                    ps_h2 = psA.tile([64, 512], f32, tag="ps_h2", space="PSUM")
                    nc.tensor.matmul(ps_h2[:], lhsT=wm2_sb[:], rhs=h1[:],
                                     start=True, stop=True)
                    h2 = mc.tile([64, 512], bf16, tag="h2")
                    nc.scalar.activation(h2[:], ps_h2[:], mybir.ActivationFunctionType.Silu)
                    for j in range(4):
                        cc = g5 * 4 + j
                        mix_chain.append(nc.tensor.matmul(
                            ps_mix[:, cc * 4:cc * 4 + 4],
                            lhsT=h2[:, j * 128:(j + 1) * 128], rhs=wm3_sb[:],
                            start=True, stop=True))
                for a, b in zip(mix_chain, mix_chain[1:]):
                    _dep(b, a)
                mix_sb = mc.tile([128, 128], f32, tag="mix_sb")
                nc.vector.tensor_copy(mix_sb[:], ps_mix[:])
                # Q build for these 32 chunks
                qs = q_sb[:, gm * 256:(gm + 1) * 256]
                mix4 = mix_sb[:].rearrange("p (c f) -> p c f", f=4)
                q8 = qs.rearrange("p (c f) -> p c f", f=8)
                a3 = aT_sb[:, gm * 96:(gm + 1) * 96].rearrange("p (c f) -> p c f", f=3)
                nc.vector.tensor_copy(q8[:, :, 0:1], mix4[:, :, 0:1])
                nc.vector.tensor_tensor(out=q8[:, :, 1:4], in0=a3[:, :, :],
                                        in1=mix4[:, :, 1:2].to_broadcast([128, 32, 3]),
                                        op=mybir.AluOpType.mult)
                nc.vector.tensor_copy(q8[:, :, 4:5], mix4[:, :, 2:3])
                nc.vector.tensor_tensor(out=q8[:, :, 5:8], in0=a3[:, :, :],
                                        in1=mix4[:, :, 3:4].to_broadcast([128, 32, 3]),
                                        op=mybir.AluOpType.mult)
            # ---- premp for ALL cores' nodes (replaces the AllGather) ------
            NR = N
            for g8 in range(NR // 1024):
                nfb = []
                for comp in range(4):
                    tb = sa2.tile([128, 1024], bf16, tag=f"nfb{comp}", name=f"nfb{comp}")
                    nc.sync.dma_start(tb[:], nfTb[comp, :, g8 * 1024:(g8 + 1) * 1024])
                    nfb.append(tb)
                f_wide = sa2.tile([128, 8, 512], bf16, tag="f_wide")
                for sub in range(8):
                    ps_f = psA.tile([128, 512], f32, tag="ps_sc", space="PSUM")
                    fchain = []
                    for comp in range(4):
                        wpr = wpreb_sb[:, 0:128] if comp == 0 else wpreb_sb[:, 128:256]
                        fchain.append(nc.tensor.matmul(
                            ps_f[:, comp * 128:(comp + 1) * 128],
                            lhsT=nfb[comp][:, sub * 128:(sub + 1) * 128],
                            rhs=wpr, start=True, stop=True))
                    for a, b in zip(fchain, fchain[1:]):
                        _dep(b, a)
                    nc.vector.tensor_copy(f_wide[:, sub, :], ps_f[:])
                nc.sync.dma_start(
                    F_full[g8 * 1024:(g8 + 1) * 1024, :].rearrange(
                        "(s p) c -> p s c", p=128),
                    f_wide[:])

        # ---------------- stage D: gather / scatter / postmp ----------------
        with tc.tile_pool(name="sd", bufs=2) as sd, \
             tc.tile_pool(name="sdh", bufs=3) as sdh, \
             tc.tile_pool(name="psagg", bufs=2, space="PSUM") as psagg, \
             tc.tile_pool(name="pso", bufs=2, space="PSUM") as pso:
            for t in range(NT):
                nch = chunks_t[t]
                c0 = off_t[t]
                fg = sd.tile([128, MAXCH, 512], bf16, tag="fg")
                oh_sl = sd.tile([128, MAXCH, 128], bf16, tag="oh_sl")
                nc.sync.dma_start(oh_sl[:, :nch, :],
                                  ohT[:, c0 * 128:(c0 + nch) * 128].rearrange(
                                      "p (j n) -> p j n", n=128))
                # dma_gather ucode handles at most 1024 indices per call
                for b0 in range(0, nch, 8):
                    bn = min(8, nch - b0)
                    nc.gpsimd.dma_gather(
                        fg[:, b0:b0 + bn, :], F_full[:],
                        eidx_sb[:, (c0 + b0) * 8:(c0 + b0 + bn) * 8],
                        bn * 128, bn * 128, 512)
                agg = psagg.tile([128, 1024], f32, tag="agg", space="PSUM")
                # PSUM accumulation groups are per 2KB bank: exactly one MM
                # per bank carries start=True (pending-zeroes the whole bank),
                # one carries stop=True, and explicit deps order them.
                bank_mms = [[], []]  # instructions per psum bank
                for j in range(nch):
                    ch = c0 + j
                    rl = rloc_sb[:, ch:ch + 1]
                    q = [q_sb[:, ch * 8 + i:ch * 8 + i + 1] for i in range(8)]
                    h = [sdh.tile([128, 128], bf16, tag=f"h{i}", name=f"h{i}")
                         for i in range(8)]
                    # DVE: 6 fused (iota==rloc)*q at 4x mode; ACT: 2 scaled
                    # copies of the host-precomputed one-hot slab
                    for i in range(6):
                        nc.vector.tensor_scalar(
                            h[i][:], iota_sb[:], rl, q[i][:],
                            mybir.AluOpType.is_equal, mybir.AluOpType.mult)
                    for i in range(6, 8):
                        nc.scalar.mul(h[i][:], oh_sl[:, j, :], q[i][:])
                    h = [t_[:] for t_ in h]
                    fs = fg[:, j, 0:128]
                    fvx = fg[:, j, 128:256]
                    fvy = fg[:, j, 256:384]
                    fvz = fg[:, j, 384:512]
                    # (column block, lhsT, rhs): block 0: s*mix0; 1: dot;
                    # 2-4: v_i*mix2; 5-7: s*a_i*mix3
                    for col, lhsT, rhs in (
                        (0, fs, h[0]), (640, fs, h[5]), (768, fs, h[6]),
                        (896, fs, h[7]), (128, fvx, h[1]), (256, fvx, h[4]),
                        (128, fvy, h[2]), (384, fvy, h[4]), (128, fvz, h[3]),
                        (512, fvz, h[4]),
                    ):
                        bank = col // 512
                        first = j == 0 and not bank_mms[bank]
                        inst = nc.tensor.matmul(
                            agg[:, col:col + 128], lhsT=lhsT, rhs=rhs,
                            start=first, stop=False)
                        bank_mms[bank].append(inst)
                for bank in range(2):
                    mms = bank_mms[bank]
                    mms[-1].ins.stop_tensor_calc = True
                    for m in mms[1:]:
                        _dep(m, mms[0])
                    for m in mms[:-1]:
                        _dep(mms[-1], m)
                agg_sb = sd.tile([128, 1024], bf16, tag="agg_sb")
                nc.scalar.copy(agg_sb[:], agg[:])
                o_ps = pso.tile([128, 512], f32, tag="o_ps", space="PSUM")
                # os = aggT0 @ Wp00 + aggT1 @ Wp01 ; ov_i = aggT(2+i) @ Wp10 + aggT(5+i) @ Wp11
                o_chain = [nc.tensor.matmul(
                    o_ps[:, 0:128], lhsT=agg_sb[:, 0:128], rhs=wp_sb[:, 0:128],
                    start=True, stop=False)]
                o_chain.append(nc.tensor.matmul(
                    o_ps[:, 0:128], lhsT=agg_sb[:, 128:256], rhs=wp_sb[:, 128:256],
                    start=False, stop=True))
                for i in range(3):
                    o_chain.append(nc.tensor.matmul(
                        o_ps[:, 128 * (i + 1):128 * (i + 2)],
                        lhsT=agg_sb[:, 128 * (2 + i):128 * (3 + i)],
                        rhs=wp_sb[:, 256:384], start=True, stop=False))
                    o_chain.append(nc.tensor.matmul(
                        o_ps[:, 128 * (i + 1):128 * (i + 2)],
                        lhsT=agg_sb[:, 128 * (5 + i):128 * (6 + i)],
                        rhs=wp_sb[:, 384:512], start=False, stop=True))
                for a, b in zip(o_chain, o_chain[1:]):
                    _dep(b, a)
                out_sb = sd.tile([128, 512], f32, tag="out_sb")
                nc.scalar.copy(
                    out_sb[:].rearrange("p (d c) -> p c d", c=4),
                    o_ps[:].rearrange("p (c d) -> p c d", c=4))
                nc.sync.dma_start(nout[t * 128:(t + 1) * 128, :], out_sb[:])

    nc.compile()
    return nc


def kernel(**inputs):
    meta, per_core, shared, unshard = _host_prep(**inputs)
    key = (meta["NSLOT"], meta["NCH"], meta["chunks_t"], meta["species_of_tile"])
    if key not in _CACHE:
        _CACHE[key] = _build(meta)
    nc = _CACHE[key]
    in_maps = [dict(pc, **shared) for pc in per_core]
    res = run_bass_kernel_spmd(nc, in_maps, core_ids=list(range(NCORES)))
    node_out = np.concatenate([res.results[k]["nout"] for k in range(NCORES)], axis=0)
    node_out = node_out.reshape(N, 128, 4)
    node_of = unshard["node_of"]
    sc = np.zeros((N, 128, 4), np.float32)
    for k in range(NCORES):
        valid = node_of[k] >= 0
        rows = res.results[k]["sc_out"][valid]
        sc[k * NSH + node_of[k][valid]] = rows.reshape(-1, 4, 128).transpose(0, 2, 1)
    return node_out, sc


# revision 33
# speedup vs baseline: 27801.7259x; 1.1866x over previous
"""Trainium2 Bass kernel for nn_AgnosticResidualInteractionBlock (GNN message passing).

Strategy (8 NeuronCores, receiver-node graph partition):
  - Host: sort edges by receiver, shard receivers 2048/core, pad each
    128-receiver tile's edge segment to whole 128-edge chunks (uniform chunk
    counts across cores), species-permute each core's node shard, fold all
    constant scales into weights, and pre-layout every per-edge quantity in
    an edge-on-partition friendly layout.
  - Device (SPMD, one Bass program on 8 cores):
      A) per-species linear (sc) for the local shard (species-grouped tiles),
         radial MLP on edges + per-edge scalar table Q, and the pre-message
         linear F for ALL nodes in bf16 (replicated compute instead of an
         AllGather: each core owns a full local F table) -- all overlapped
         across TensorE / ACT / DVE / three DMA queues.
      D) main loop per node tile: dma_gather sender rows of F (<=1024 idx per
         call, a ucode limit), build scaled one-hot scatter matrices (fused
         is_equal*q tensor_scalar on DVE at 4x + scaled copies of a
         host-precomputed one-hot slab on ACT), scatter-accumulate aggT in
         PSUM via TensorE (one accumulation group per 2KB PSUM bank, ordered
         by explicit deps), then the post-message linear and the interleaved
         output eviction.
  - Host: concatenate/unpermute shard outputs.
"""
import sys
import numpy as np

try:
    import concourse.bacc as bacc
except ImportError:  # pragma: no cover
    sys.path.insert(0, "/opt/trn_rl_repo")
    import concourse.bacc as bacc

import ml_dtypes
from contextlib import ExitStack

import concourse.bass as bass
import concourse.mybir as mybir
import concourse.tile as tile
from concourse import library_config
from concourse.bass_utils import run_bass_kernel_spmd

BF16 = ml_dtypes.bfloat16
N, C, E, S = 16384, 128, 262144, 10
NCORES = 8
NSH = N // NCORES            # 2048 nodes per core
NT = NSH // 128              # 16 node tiles per core
AVG = 16.0
INV_C = 1.0 / np.sqrt(C)
INV2C = 1.0 / np.sqrt(2 * C)
INV_SQRT3 = 1.0 / np.sqrt(3.0)

_CACHE = {}


def _host_prep(node_specie, node_feats, edge_attrs, edge_feats, senders, receivers,
               W_sc0, W_sc1, W_pre0, W_pre1, W_mlp1, W_mlp2, W_mlp3, W_post0, W_post1):
    """All numpy. Returns (meta, per_core_inputs, shared_inputs, unshard_info)."""
    senders = np.asarray(senders).astype(np.int64)
    receivers = np.asarray(receivers).astype(np.int64)
    node_specie = np.asarray(node_specie).astype(np.int64)
    node_feats = np.asarray(node_feats, dtype=np.float32)
    edge_attrs = np.asarray(edge_attrs, dtype=np.float32)
    edge_feats = np.asarray(edge_feats, dtype=np.float32)

    # ---- species permutation per core, uniform tile->species map -----------
    spec_counts = np.zeros((NCORES, S), np.int64)
    core_of_node = np.arange(N) // NSH
    for k in range(NCORES):
        sp = node_specie[k * NSH:(k + 1) * NSH]
        spec_counts[k] = np.bincount(sp, minlength=S)
    tiles_per_spec = np.maximum(1, -(-spec_counts.max(axis=0) // 128))  # ceil
    NSLOT = int(tiles_per_spec.sum() * 128)
    spec_tile_off = np.concatenate([[0], np.cumsum(tiles_per_spec)])[:-1] * 128
    species_of_tile = np.repeat(np.arange(S), tiles_per_spec)

    # slot_of[k, local_node] ; node_of[k, slot] (-1 = dummy)
    slot_of = np.zeros((NCORES, NSH), np.int64)
    node_of = -np.ones((NCORES, NSLOT), np.int64)
    for k in range(NCORES):
        sp = node_specie[k * NSH:(k + 1) * NSH]
        for s in range(S):
            loc = np.nonzero(sp == s)[0]
            slots = spec_tile_off[s] + np.arange(len(loc))
            slot_of[k, loc] = slots
            node_of[k, slots] = loc  # local node id
    assert N < 32768

    # ---- edge sort by receiver, tile segmentation, padding -----------------
    order = np.argsort(receivers, kind="stable")
    s_sorted = senders[order]
    r_sorted = receivers[order]
    a_sorted = edge_attrs[order]
    ef_sorted = edge_feats[order]
    # counts per (core, tile-index)
    gtile = r_sorted // 128  # 0..127
    cnt = np.bincount(gtile, minlength=128).reshape(NCORES, NT)
    chunks_t = np.maximum(1, -(-cnt.max(axis=0) // 128))  # [NT]
    NCH = int(chunks_t.sum())
    NCHP = -(-NCH // 32) * 32
    EPAD = NCH * 128
    EPADP = NCHP * 128
    off_t = np.concatenate([[0], np.cumsum(chunks_t)])[:-1]  # chunk offsets

    # per-core padded edge stream arrays
    eidx = np.zeros((NCORES, EPADP), np.int64)      # F row per edge
    rloc = -np.ones((NCORES, EPADP), np.float32)    # local receiver 0..127
    aarr = np.zeros((NCORES, EPADP, 3), np.float32)
    efarr = np.zeros((NCORES, EPADP, 8), np.float32)
    gstart = np.concatenate([[0], np.cumsum(cnt.reshape(-1))])
    for k in range(NCORES):
        for t in range(NT):
            g = k * NT + t
            seg = slice(gstart[g], gstart[g + 1])
            n = gstart[g + 1] - gstart[g]
            base = int(off_t[t]) * 128
            eidx[k, base:base + n] = s_sorted[seg]
            rloc[k, base:base + n] = (r_sorted[seg] % 128).astype(np.float32)
            aarr[k, base:base + n] = a_sorted[seg, 1:4]
            efarr[k, base:base + n] = ef_sorted[seg]

    # ---- weights with folded scales ---------------------------------------
    wsc0T = (np.asarray(W_sc0, np.float32) * INV_C).transpose(1, 0, 2).reshape(128, S * 128)
    wsc1T = (np.asarray(W_sc1, np.float32) * INV_C).transpose(1, 0, 2).reshape(128, S * 128)
    wpre = np.concatenate([np.asarray(W_pre0, np.float32) * INV_C,
                           np.asarray(W_pre1, np.float32) * INV_C], axis=1)  # [128, 256]
    wm1 = (np.asarray(W_mlp1, np.float32) / np.sqrt(8.0)).astype(BF16)
    wm2 = (np.asarray(W_mlp2, np.float32) * 0.125).astype(BF16)
    wm3 = (np.asarray(W_mlp3, np.float32) * 0.125).astype(BF16)
    sc_post = INV2C / AVG
    wp = np.concatenate([
        np.asarray(W_post0, np.float32)[:, 0, :] * sc_post,
        np.asarray(W_post0, np.float32)[:, 1, :] * sc_post * INV_SQRT3,
        np.asarray(W_post1, np.float32)[:, 0, :] * sc_post,
        np.asarray(W_post1, np.float32)[:, 1, :] * sc_post,
    ], axis=1).astype(BF16)  # [128, 512]

    # ---- per-core device arrays -------------------------------------------
    per_core = []
    for k in range(NCORES):
        nf = node_feats[k * NSH:(k + 1) * NSH]  # [2048, 128, 4]
        nfT = np.zeros((4, 128, NSLOT), np.float32)
        valid = node_of[k] >= 0
        nfT[:, :, valid] = nf[node_of[k][valid]].transpose(2, 1, 0)
        # idx wrap for dma_gather: idx i -> partition i%16, col i//16, tile 8x
        iw = eidx[k].reshape(EPADP // 16, 16).T.astype(np.int16)
        iwr = np.tile(iw, (8, 1))
        rlocT = np.ascontiguousarray(
            rloc[k].reshape(NCHP, 128).T).astype(np.float32)  # [128, NCHP]
        ohT = (rloc[k].reshape(NCHP, 128, 1) ==
               np.arange(128, dtype=np.float32)).astype(BF16)
        ohT = np.ascontiguousarray(ohT.transpose(1, 0, 2).reshape(128, NCHP * 128))
        aT = np.ascontiguousarray(
            aarr[k].reshape(NCHP, 128, 3).transpose(1, 0, 2).reshape(128, NCHP * 3))
        efT = np.ascontiguousarray(efarr[k].reshape(EPADP, 8).T).astype(BF16)
        per_core.append(dict(
            nfT=nfT, eidx=iwr, rlocT=rlocT, aT=aT.astype(np.float32), efT=efT,
            ohT=ohT,
        ))

    iota = np.tile(np.arange(128, dtype=np.float32), (128, 1)).astype(BF16)
    nfTb = np.ascontiguousarray(node_feats.transpose(2, 1, 0)).astype(BF16)
    shared = dict(wsc0T=wsc0T, wsc1T=wsc1T, wpre=wpre, wpre_b=wpre.astype(BF16),
                  nfTb=nfTb, wm1=wm1, wm2=wm2, wm3=wm3, wp=wp, iota=iota)
    meta = dict(NSLOT=NSLOT, NCH=NCH, NCHP=NCHP, EPADP=EPADP,
                chunks_t=tuple(int(x) for x in chunks_t),
                off_t=tuple(int(x) for x in off_t),
                species_of_tile=tuple(int(x) for x in species_of_tile))
    unshard = dict(node_of=node_of, NSLOT=NSLOT)
    return meta, per_core, shared, unshard


def _dep(later, earlier):
    tile.add_dep_helper(later.ins, earlier.ins, sync=False, reason="psum group order")


def _build(meta):
    NSLOT = meta["NSLOT"]
    NCH, NCHP, EPADP = meta["NCH"], meta["NCHP"], meta["EPADP"]
    chunks_t, off_t = meta["chunks_t"], meta["off_t"]
    species_of_tile = meta["species_of_tile"]
    NGT = NSLOT // 128
    MAXCH = max(chunks_t)
    f32, bf16, i16 = mybir.dt.float32, mybir.dt.bfloat16, mybir.dt.int16

    nc = bacc.Bacc("TRN2", target_bir_lowering=False)
    # inputs
    nfT = nc.dram_tensor("nfT", [4, 128, NSLOT], f32, kind="ExternalInput")
    eidx = nc.dram_tensor("eidx", [128, EPADP // 16], i16, kind="ExternalInput")
    rlocT = nc.dram_tensor("rlocT", [128, NCHP], f32, kind="ExternalInput")
    aT = nc.dram_tensor("aT", [128, NCHP * 3], f32, kind="ExternalInput")
    ohT = nc.dram_tensor("ohT", [128, NCHP * 128], bf16, kind="ExternalInput")
    efT = nc.dram_tensor("efT", [8, EPADP], bf16, kind="ExternalInput")
    wsc0T = nc.dram_tensor("wsc0T", [128, S * 128], f32, kind="ExternalInput")
    wsc1T = nc.dram_tensor("wsc1T", [128, S * 128], f32, kind="ExternalInput")
    wpre = nc.dram_tensor("wpre", [128, 256], f32, kind="ExternalInput")
    wpre_b = nc.dram_tensor("wpre_b", [128, 256], bf16, kind="ExternalInput")
    nfTb = nc.dram_tensor("nfTb", [4, 128, N], bf16, kind="ExternalInput")
    wm1 = nc.dram_tensor("wm1", [8, 64], bf16, kind="ExternalInput")
    wm2 = nc.dram_tensor("wm2", [64, 64], bf16, kind="ExternalInput")
    wm3 = nc.dram_tensor("wm3", [64, 4], bf16, kind="ExternalInput")
    wp = nc.dram_tensor("wp", [128, 512], bf16, kind="ExternalInput")
    iota = nc.dram_tensor("iota", [128, 128], bf16, kind="ExternalInput")
    # outputs
    sc_out = nc.dram_tensor("sc_out", [NSLOT, 512], f32, kind="ExternalOutput")
    nout = nc.dram_tensor("nout", [NSH, 512], f32, kind="ExternalOutput")

    with tile.TileContext(nc) as tc, ExitStack() as ctx:
        nc.gpsimd.load_library(library_config.mlp)
        dr = ctx.enter_context(tc.tile_pool(name="dr", bufs=1, space="DRAM"))
        F_full = dr.tile([N, 512], bf16)

        cst = ctx.enter_context(tc.tile_pool(name="cst", bufs=1))
        iota_sb = cst.tile([128, 128], bf16)
        nc.sync.dma_start(iota_sb[:], iota[:])
        wp_sb = cst.tile([128, 512], bf16)
        nc.sync.dma_start(wp_sb[:], wp[:])
        wm1_sb = cst.tile([8, 64], bf16)
        nc.sync.dma_start(wm1_sb[:], wm1[:])
        wm2_sb = cst.tile([64, 64], bf16)
        nc.sync.dma_start(wm2_sb[:], wm2[:])
        wm3_sb = cst.tile([64, 4], bf16)
        nc.sync.dma_start(wm3_sb[:], wm3[:])
        eidx_sb = cst.tile([128, EPADP // 16], i16)
        nc.sync.dma_start(eidx_sb[:], eidx[:])
        rloc_sb = cst.tile([128, NCHP], f32)
        nc.sync.dma_start(rloc_sb[:], rlocT[:])
        aT_sb = cst.tile([128, NCHP * 3], f32)
        nc.sync.dma_start(aT_sb[:], aT[:])
        q_sb = cst.tile([128, NCHP * 8], f32)

        # ---- stages A (sc), C (MLP+Q), then premp-all; one PSUM scope ------
        with tc.tile_pool(name="sa", bufs=1) as sa, \
             tc.tile_pool(name="sa2", bufs=3) as sa2, \
             tc.tile_pool(name="mc", bufs=3) as mc, \
             tc.tile_pool(name="psA", bufs=2, space="PSUM") as psA:
            wsc0_sb = sa.tile([128, S * 128], f32)
            nc.sync.dma_start(wsc0_sb[:], wsc0T[:])
            wsc1_sb = sa.tile([128, S * 128], f32)
            nc.sync.dma_start(wsc1_sb[:], wsc1T[:])
            ef_sb = sa.tile([8, EPADP], bf16)
            nc.sync.dma_start(ef_sb[:], efT[:])
            wpreb_sb = sa.tile([128, 256], bf16)
            nc.sync.dma_start(wpreb_sb[:], wpre_b[:])
            nfc = []
            for comp in range(4):
                t = sa.tile([128, NSLOT], f32, name=f"nfc{comp}")
                nc.gpsimd.dma_start(t[:], nfT[comp, :, :])
                nfc.append(t)
            # ---- A: per-species sc for the local shard ----
            for gt in range(NGT):
                sp = species_of_tile[gt]
                ps_sc = psA.tile([128, 512], f32, tag="ps_sc", space="PSUM")
                mm_chain = []
                for comp in range(4):
                    lhsT = nfc[comp][:, gt * 128:(gt + 1) * 128]
                    wsc = (wsc0_sb if comp == 0 else wsc1_sb)[:, sp * 128:(sp + 1) * 128]
                    mm_chain.append(nc.tensor.matmul(
                        ps_sc[:, comp * 128:(comp + 1) * 128],
                        lhsT=lhsT, rhs=wsc, start=True, stop=True))
                for a, b in zip(mm_chain, mm_chain[1:]):
                    _dep(b, a)
                sc_sb = sa2.tile([128, 512], f32, tag="sc_sb")
                nc.vector.tensor_copy(sc_sb[:], ps_sc[:])
                nc.gpsimd.dma_start(sc_out[gt * 128:(gt + 1) * 128, :], sc_sb[:])
            # ---- C: radial MLP + per-edge scalar table Q ----
            for gm in range(NCHP // 32):   # 32 chunks = 4096 edges per group
                ps_mix = psA.tile([128, 128], f32, tag="ps_mix", space="PSUM")
                mix_chain = []
                for g5 in range(8):        # 512-edge subgroups
                    e0 = gm * 4096 + g5 * 512
                    ps_h = psA.tile([64, 512], f32, tag="ps_h", space="PSUM")
                    nc.tensor.matmul(ps_h[:], lhsT=wm1_sb[:], rhs=ef_sb[:, e0:e0 + 512],
                                     start=True, stop=True)
                    h1 = mc.tile([64, 512], bf16, tag="h1")
                    nc.scalar.activation(h1[:], ps_h[:], mybir.ActivationFunctionType.Silu)
                    ps_h2 = psA.tile([64, 512], f32, tag="ps_h2", space="PSUM")
                    nc.tensor.matmul(ps_h2[:], lhsT=wm2_sb[:], rhs=h1[:],
                                     start=True, stop=True)
                    h2 = mc.tile([64, 512], bf16, tag="h2")
                    nc.scalar.activation(h2[:], ps_h2[:], mybir.ActivationFunctionType.Silu)
                    for j in range(4):
                        cc = g5 * 4 + j
                        mix_chain.append(nc.tensor.matmul(
                            ps_mix[:, cc * 4:cc * 4 + 4],
                            lhsT=h2[:, j * 128:(j + 1) * 128], rhs=wm3_sb[:],
                            start=True, stop=True))
                for a, b in zip(mix_chain, mix_chain[1:]):
                    _dep(b, a)
                mix_sb = mc.tile([128, 128], f32, tag="mix_sb")
                nc.vector.tensor_copy(mix_sb[:], ps_mix[:])
                # Q build for these 32 chunks
                qs = q_sb[:, gm * 256:(gm + 1) * 256]
                mix4 = mix_sb[:].rearrange("p (c f) -> p c f", f=4)
                q8 = qs.rearrange("p (c f) -> p c f", f=8)
                a3 = aT_sb[:, gm * 96:(gm + 1) * 96].rearrange("p (c f) -> p c f", f=3)
                nc.vector.tensor_copy(q8[:, :, 0:1], mix4[:, :, 0:1])
                nc.vector.tensor_tensor(out=q8[:, :, 1:4], in0=a3[:, :, :],
                                        in1=mix4[:, :, 1:2].to_broadcast([128, 32, 3]),
                                        op=mybir.AluOpType.mult)
                nc.vector.tensor_copy(q8[:, :, 4:5], mix4[:, :, 2:3])
                nc.vector.tensor_tensor(out=q8[:, :, 5:8], in0=a3[:, :, :],
                                        in1=mix4[:, :, 3:4].to_broadcast([128, 32, 3]),
                                        op=mybir.AluOpType.mult)
            # ---- premp for ALL cores' nodes (replaces the AllGather) ------
            NR = N
            for g8 in range(NR // 1024):
                nfb = []
                for comp in range(4):
                    tb = sa2.tile([128, 1024], bf16, tag=f"nfb{comp}", name=f"nfb{comp}")
                    nc.gpsimd.dma_start(tb[:], nfTb[comp, :, g8 * 1024:(g8 + 1) * 1024])
                    nfb.append(tb)
                f_wide = sa2.tile([128, 8, 512], bf16, tag="f_wide")
                for sub in range(8):
                    ps_f = psA.tile([128, 512], f32, tag="ps_sc", space="PSUM")
                    fchain = []
                    for comp in range(4):
                        wpr = wpreb_sb[:, 0:128] if comp == 0 else wpreb_sb[:, 128:256]
                        fchain.append(nc.tensor.matmul(
                            ps_f[:, comp * 128:(comp + 1) * 128],
                            lhsT=nfb[comp][:, sub * 128:(sub + 1) * 128],
                            rhs=wpr, start=True, stop=True))
                    for a, b in zip(fchain, fchain[1:]):
                        _dep(b, a)
                    nc.vector.tensor_copy(f_wide[:, sub, :], ps_f[:])
                nc.sync.dma_start(
                    F_full[g8 * 1024:(g8 + 1) * 1024, :].rearrange(
                        "(s p) c -> p s c", p=128),
                    f_wide[:])

        # ---------------- stage D: gather / scatter / postmp ----------------
        with tc.tile_pool(name="sd", bufs=2) as sd, \
             tc.tile_pool(name="sdh", bufs=3) as sdh, \
             tc.tile_pool(name="psagg", bufs=2, space="PSUM") as psagg, \
             tc.tile_pool(name="pso", bufs=2, space="PSUM") as pso:
            for t in range(NT):
                nch = chunks_t[t]
                c0 = off_t[t]
                fg = sd.tile([128, MAXCH, 512], bf16, tag="fg")
                oh_sl = sd.tile([128, MAXCH, 128], bf16, tag="oh_sl")
                nc.sync.dma_start(oh_sl[:, :nch, :],
                                  ohT[:, c0 * 128:(c0 + nch) * 128].rearrange(
                                      "p (j n) -> p j n", n=128))
                # dma_gather ucode handles at most 1024 indices per call
                for b0 in range(0, nch, 8):
                    bn = min(8, nch - b0)
                    nc.gpsimd.dma_gather(
                        fg[:, b0:b0 + bn, :], F_full[:],
                        eidx_sb[:, (c0 + b0) * 8:(c0 + b0 + bn) * 8],
                        bn * 128, bn * 128, 512)
                agg = psagg.tile([128, 1024], f32, tag="agg", space="PSUM")
                # PSUM accumulation groups are per 2KB bank: exactly one MM
                # per bank carries start=True (pending-zeroes the whole bank),
                # one carries stop=True, and explicit deps order them.
                bank_mms = [[], []]  # instructions per psum bank
                for j in range(nch):
                    ch = c0 + j
                    rl = rloc_sb[:, ch:ch + 1]
                    q = [q_sb[:, ch * 8 + i:ch * 8 + i + 1] for i in range(8)]
                    h = [sdh.tile([128, 128], bf16, tag=f"h{i}", name=f"h{i}")
                         for i in range(8)]
                    # DVE: fused (iota==rloc)*q at 4x mode; ACT: scaled
                    # copies of the host-precomputed one-hot slab.  Alternate
                    # 7/1 and 6/2 splits to balance the two engines.
                    ndve = 7 if j % 2 == 0 else 6
                    for i in range(ndve):
                        nc.vector.tensor_scalar(
                            h[i][:], iota_sb[:], rl, q[i][:],
                            mybir.AluOpType.is_equal, mybir.AluOpType.mult)
                    for i in range(ndve, 8):
                        nc.scalar.mul(h[i][:], oh_sl[:, j, :], q[i][:])
                    h = [t_[:] for t_ in h]
                    fs = fg[:, j, 0:128]
                    fvx = fg[:, j, 128:256]
                    fvy = fg[:, j, 256:384]
                    fvz = fg[:, j, 384:512]
                    # (column block, lhsT, rhs): block 0: s*mix0; 1: dot;
                    # 2-4: v_i*mix2; 5-7: s*a_i*mix3
                    for col, lhsT, rhs in (
                        (0, fs, h[0]), (640, fs, h[5]), (768, fs, h[6]),
                        (896, fs, h[7]), (128, fvx, h[1]), (256, fvx, h[4]),
                        (128, fvy, h[2]), (384, fvy, h[4]), (128, fvz, h[3]),
                        (512, fvz, h[4]),
                    ):
                        bank = col // 512
                        first = j == 0 and not bank_mms[bank]
                        inst = nc.tensor.matmul(
                            agg[:, col:col + 128], lhsT=lhsT, rhs=rhs,
                            start=first, stop=False)
                        bank_mms[bank].append(inst)
                for bank in range(2):
                    mms = bank_mms[bank]
                    mms[-1].ins.stop_tensor_calc = True
                    for m in mms[1:]:
                        _dep(m, mms[0])
                    for m in mms[:-1]:
                        _dep(mms[-1], m)
                agg_sb = sd.tile([128, 1024], bf16, tag="agg_sb")
                nc.scalar.copy(agg_sb[:], agg[:])
                o_ps = pso.tile([128, 512], f32, tag="o_ps", space="PSUM")
                # os = aggT0 @ Wp00 + aggT1 @ Wp01 ; ov_i = aggT(2+i) @ Wp10 + aggT(5+i) @ Wp11
                o_chain = [nc.tensor.matmul(
                    o_ps[:, 0:128], lhsT=agg_sb[:, 0:128], rhs=wp_sb[:, 0:128],
                    start=True, stop=False)]
                o_chain.append(nc.tensor.matmul(
                    o_ps[:, 0:128], lhsT=agg_sb[:, 128:256], rhs=wp_sb[:, 128:256],
                    start=False, stop=True))
                for i in range(3):
                    o_chain.append(nc.tensor.matmul(
                        o_ps[:, 128 * (i + 1):128 * (i + 2)],
                        lhsT=agg_sb[:, 128 * (2 + i):128 * (3 + i)],
                        rhs=wp_sb[:, 256:384], start=True, stop=False))
                    o_chain.append(nc.tensor.matmul(
                        o_ps[:, 128 * (i + 1):128 * (i + 2)],
                        lhsT=agg_sb[:, 128 * (5 + i):128 * (6 + i)],
                        rhs=wp_sb[:, 384:512], start=False, stop=True))
                for a, b in zip(o_chain, o_chain[1:]):
                    _dep(b, a)
                out_sb = sd.tile([128, 512], f32, tag="out_sb")
                nc.scalar.copy(
                    out_sb[:].rearrange("p (d c) -> p c d", c=4),
                    o_ps[:].rearrange("p (c d) -> p c d", c=4))
                nc.sync.dma_start(nout[t * 128:(t + 1) * 128, :], out_sb[:])

    nc.compile()
    return nc


def kernel(**inputs):
    meta, per_core, shared, unshard = _host_prep(**inputs)
    key = (meta["NSLOT"], meta["NCH"], meta["chunks_t"], meta["species_of_tile"])
    if key not in _CACHE:
        _CACHE[key] = _build(meta)
    nc = _CACHE[key]
    in_maps = [dict(pc, **shared) for pc in per_core]
    res = run_bass_kernel_spmd(nc, in_maps, core_ids=list(range(NCORES)))
    node_out = np.concatenate([res.results[k]["nout"] for k in range(NCORES)], axis=0)
    node_out = node_out.reshape(N, 128, 4)
    node_of = unshard["node_of"]
    sc = np.zeros((N, 128, 4), np.float32)
    for k in range(NCORES):
        valid = node_of[k] >= 0
        rows = res.results[k]["sc_out"][valid]
        sc[k * NSH + node_of[k][valid]] = rows.reshape(-1, 4, 128).transpose(0, 2, 1)
    return node_out, sc


# revision 35
# speedup vs baseline: 27859.6577x; 1.0021x over previous
"""Trainium2 Bass kernel for nn_AgnosticResidualInteractionBlock (GNN message passing).

Strategy (8 NeuronCores, receiver-node graph partition):
  - Host: sort edges by receiver, shard receivers 2048/core, pad each
    128-receiver tile's edge segment to whole 128-edge chunks (uniform chunk
    counts across cores), species-permute each core's node shard, fold all
    constant scales into weights, and pre-layout every per-edge quantity in
    an edge-on-partition friendly layout.
  - Device (SPMD, one Bass program on 8 cores):
      A) per-species linear (sc) for the local shard (species-grouped tiles),
         radial MLP on edges + per-edge scalar table Q, and the pre-message
         linear F for ALL nodes in bf16 (replicated compute instead of an
         AllGather: each core owns a full local F table) -- all overlapped
         across TensorE / ACT / DVE / three DMA queues.
      D) main loop per node tile: dma_gather sender rows of F (<=1024 idx per
         call, a ucode limit), build scaled one-hot scatter matrices (fused
         is_equal*q tensor_scalar on DVE at 4x + scaled copies of a
         host-precomputed one-hot slab on ACT), scatter-accumulate aggT in
         PSUM via TensorE (one accumulation group per 2KB PSUM bank, ordered
         by explicit deps), then the post-message linear and the interleaved
         output eviction.
  - Host: concatenate/unpermute shard outputs.
"""
import sys
import numpy as np

try:
    import concourse.bacc as bacc
except ImportError:  # pragma: no cover
    sys.path.insert(0, "/opt/trn_rl_repo")
    import concourse.bacc as bacc

import ml_dtypes
from contextlib import ExitStack

import concourse.bass as bass
import concourse.mybir as mybir
import concourse.tile as tile
from concourse import library_config
from concourse.bass_utils import run_bass_kernel_spmd

BF16 = ml_dtypes.bfloat16
N, C, E, S = 16384, 128, 262144, 10
NCORES = 8
NSH = N // NCORES            # 2048 nodes per core
NT = NSH // 128              # 16 node tiles per core
AVG = 16.0
INV_C = 1.0 / np.sqrt(C)
INV2C = 1.0 / np.sqrt(2 * C)
INV_SQRT3 = 1.0 / np.sqrt(3.0)

_CACHE = {}


def _host_prep(node_specie, node_feats, edge_attrs, edge_feats, senders, receivers,
               W_sc0, W_sc1, W_pre0, W_pre1, W_mlp1, W_mlp2, W_mlp3, W_post0, W_post1):
    """All numpy. Returns (meta, per_core_inputs, shared_inputs, unshard_info)."""
    senders = np.asarray(senders).astype(np.int64)
    receivers = np.asarray(receivers).astype(np.int64)
    node_specie = np.asarray(node_specie).astype(np.int64)
    node_feats = np.asarray(node_feats, dtype=np.float32)
    edge_attrs = np.asarray(edge_attrs, dtype=np.float32)
    edge_feats = np.asarray(edge_feats, dtype=np.float32)

    # ---- species permutation per core, uniform tile->species map -----------
    spec_counts = np.zeros((NCORES, S), np.int64)
    core_of_node = np.arange(N) // NSH
    for k in range(NCORES):
        sp = node_specie[k * NSH:(k + 1) * NSH]
        spec_counts[k] = np.bincount(sp, minlength=S)
    tiles_per_spec = np.maximum(1, -(-spec_counts.max(axis=0) // 128))  # ceil
    NSLOT = int(tiles_per_spec.sum() * 128)
    spec_tile_off = np.concatenate([[0], np.cumsum(tiles_per_spec)])[:-1] * 128
    species_of_tile = np.repeat(np.arange(S), tiles_per_spec)

    # slot_of[k, local_node] ; node_of[k, slot] (-1 = dummy)
    slot_of = np.zeros((NCORES, NSH), np.int64)
    node_of = -np.ones((NCORES, NSLOT), np.int64)
    for k in range(NCORES):
        sp = node_specie[k * NSH:(k + 1) * NSH]
        for s in range(S):
            loc = np.nonzero(sp == s)[0]
            slots = spec_tile_off[s] + np.arange(len(loc))
            slot_of[k, loc] = slots
            node_of[k, slots] = loc  # local node id
    assert N < 32768

    # ---- edge sort by receiver, tile segmentation, padding -----------------
    order = np.argsort(receivers, kind="stable")
    s_sorted = senders[order]
    r_sorted = receivers[order]
    a_sorted = edge_attrs[order]
    ef_sorted = edge_feats[order]
    # counts per (core, tile-index)
    gtile = r_sorted // 128  # 0..127
    cnt = np.bincount(gtile, minlength=128).reshape(NCORES, NT)
    chunks_t = np.maximum(1, -(-cnt.max(axis=0) // 128))  # [NT]
    NCH = int(chunks_t.sum())
    NCHP = -(-NCH // 32) * 32
    EPAD = NCH * 128
    EPADP = NCHP * 128
    off_t = np.concatenate([[0], np.cumsum(chunks_t)])[:-1]  # chunk offsets

    # per-core padded edge stream arrays
    eidx = np.zeros((NCORES, EPADP), np.int64)      # F row per edge
    rloc = -np.ones((NCORES, EPADP), np.float32)    # local receiver 0..127
    aarr = np.zeros((NCORES, EPADP, 3), np.float32)
    efarr = np.zeros((NCORES, EPADP, 8), np.float32)
    gstart = np.concatenate([[0], np.cumsum(cnt.reshape(-1))])
    for k in range(NCORES):
        for t in range(NT):
            g = k * NT + t
            seg = slice(gstart[g], gstart[g + 1])
            n = gstart[g + 1] - gstart[g]
            base = int(off_t[t]) * 128
            eidx[k, base:base + n] = s_sorted[seg]
            rloc[k, base:base + n] = (r_sorted[seg] % 128).astype(np.float32)
            aarr[k, base:base + n] = a_sorted[seg, 1:4]
            efarr[k, base:base + n] = ef_sorted[seg]

    # ---- weights with folded scales ---------------------------------------
    wsc0T = (np.asarray(W_sc0, np.float32) * INV_C).transpose(1, 0, 2).reshape(128, S * 128)
    wsc1T = (np.asarray(W_sc1, np.float32) * INV_C).transpose(1, 0, 2).reshape(128, S * 128)
    wpre = np.concatenate([np.asarray(W_pre0, np.float32) * INV_C,
                           np.asarray(W_pre1, np.float32) * INV_C], axis=1)  # [128, 256]
    wm1 = (np.asarray(W_mlp1, np.float32) / np.sqrt(8.0)).astype(BF16)
    wm2 = (np.asarray(W_mlp2, np.float32) * 0.125).astype(BF16)
    wm3 = (np.asarray(W_mlp3, np.float32) * 0.125).astype(BF16)
    sc_post = INV2C / AVG
    wp = np.concatenate([
        np.asarray(W_post0, np.float32)[:, 0, :] * sc_post,
        np.asarray(W_post0, np.float32)[:, 1, :] * sc_post * INV_SQRT3,
        np.asarray(W_post1, np.float32)[:, 0, :] * sc_post,
        np.asarray(W_post1, np.float32)[:, 1, :] * sc_post,
    ], axis=1).astype(BF16)  # [128, 512]

    # ---- per-core device arrays -------------------------------------------
    per_core = []
    for k in range(NCORES):
        nf = node_feats[k * NSH:(k + 1) * NSH]  # [2048, 128, 4]
        nfT = np.zeros((4, 128, NSLOT), np.float32)
        valid = node_of[k] >= 0
        nfT[:, :, valid] = nf[node_of[k][valid]].transpose(2, 1, 0)
        # idx wrap for dma_gather: idx i -> partition i%16, col i//16, tile 8x
        iw = eidx[k].reshape(EPADP // 16, 16).T.astype(np.int16)
        iwr = np.tile(iw, (8, 1))
        rlocT = np.ascontiguousarray(
            rloc[k].reshape(NCHP, 128).T).astype(np.float32)  # [128, NCHP]
        ohT = (rloc[k].reshape(NCHP, 128, 1) ==
               np.arange(128, dtype=np.float32)).astype(BF16)
        ohT = np.ascontiguousarray(ohT.transpose(1, 0, 2).reshape(128, NCHP * 128))
        aT = np.ascontiguousarray(
            aarr[k].reshape(NCHP, 128, 3).transpose(1, 0, 2).reshape(128, NCHP * 3))
        efT = np.ascontiguousarray(efarr[k].reshape(EPADP, 8).T).astype(BF16)
        per_core.append(dict(
            nfT=nfT, eidx=iwr, rlocT=rlocT, aT=aT.astype(np.float32), efT=efT,
            ohT=ohT,
        ))

    iota = np.tile(np.arange(128, dtype=np.float32), (128, 1)).astype(BF16)
    nfTb = np.ascontiguousarray(node_feats.transpose(2, 1, 0)).astype(BF16)
    shared = dict(wsc0T=wsc0T, wsc1T=wsc1T, wpre=wpre, wpre_b=wpre.astype(BF16),
                  nfTb=nfTb, wm1=wm1, wm2=wm2, wm3=wm3, wp=wp, iota=iota)
    meta = dict(NSLOT=NSLOT, NCH=NCH, NCHP=NCHP, EPADP=EPADP,
                chunks_t=tuple(int(x) for x in chunks_t),
                off_t=tuple(int(x) for x in off_t),
                species_of_tile=tuple(int(x) for x in species_of_tile))
    unshard = dict(node_of=node_of, NSLOT=NSLOT)
    return meta, per_core, shared, unshard


def _dep(later, earlier):
    tile.add_dep_helper(later.ins, earlier.ins, sync=False, reason="psum group order")


def _build(meta):
    NSLOT = meta["NSLOT"]
    NCH, NCHP, EPADP = meta["NCH"], meta["NCHP"], meta["EPADP"]
    chunks_t, off_t = meta["chunks_t"], meta["off_t"]
    species_of_tile = meta["species_of_tile"]
    NGT = NSLOT // 128
    MAXCH = max(chunks_t)
    f32, bf16, i16 = mybir.dt.float32, mybir.dt.bfloat16, mybir.dt.int16

    nc = bacc.Bacc("TRN2", target_bir_lowering=False)
    # inputs
    nfT = nc.dram_tensor("nfT", [4, 128, NSLOT], f32, kind="ExternalInput")
    eidx = nc.dram_tensor("eidx", [128, EPADP // 16], i16, kind="ExternalInput")
    rlocT = nc.dram_tensor("rlocT", [128, NCHP], f32, kind="ExternalInput")
    aT = nc.dram_tensor("aT", [128, NCHP * 3], f32, kind="ExternalInput")
    ohT = nc.dram_tensor("ohT", [128, NCHP * 128], bf16, kind="ExternalInput")
    efT = nc.dram_tensor("efT", [8, EPADP], bf16, kind="ExternalInput")
    wsc0T = nc.dram_tensor("wsc0T", [128, S * 128], f32, kind="ExternalInput")
    wsc1T = nc.dram_tensor("wsc1T", [128, S * 128], f32, kind="ExternalInput")
    wpre = nc.dram_tensor("wpre", [128, 256], f32, kind="ExternalInput")
    wpre_b = nc.dram_tensor("wpre_b", [128, 256], bf16, kind="ExternalInput")
    nfTb = nc.dram_tensor("nfTb", [4, 128, N], bf16, kind="ExternalInput")
    wm1 = nc.dram_tensor("wm1", [8, 64], bf16, kind="ExternalInput")
    wm2 = nc.dram_tensor("wm2", [64, 64], bf16, kind="ExternalInput")
    wm3 = nc.dram_tensor("wm3", [64, 4], bf16, kind="ExternalInput")
    wp = nc.dram_tensor("wp", [128, 512], bf16, kind="ExternalInput")
    iota = nc.dram_tensor("iota", [128, 128], bf16, kind="ExternalInput")
    # outputs
    sc_out = nc.dram_tensor("sc_out", [NSLOT, 512], f32, kind="ExternalOutput")
    nout = nc.dram_tensor("nout", [NSH, 512], f32, kind="ExternalOutput")

    with tile.TileContext(nc) as tc, ExitStack() as ctx:
        nc.gpsimd.load_library(library_config.mlp)
        dr = ctx.enter_context(tc.tile_pool(name="dr", bufs=1, space="DRAM"))
        F_full = dr.tile([N, 512], bf16)

        cst = ctx.enter_context(tc.tile_pool(name="cst", bufs=1))
        iota_sb = cst.tile([128, 128], bf16)
        nc.sync.dma_start(iota_sb[:], iota[:])
        wp_sb = cst.tile([128, 512], bf16)
        nc.sync.dma_start(wp_sb[:], wp[:])
        wm1_sb = cst.tile([8, 64], bf16)
        nc.sync.dma_start(wm1_sb[:], wm1[:])
        wm2_sb = cst.tile([64, 64], bf16)
        nc.sync.dma_start(wm2_sb[:], wm2[:])
        wm3_sb = cst.tile([64, 4], bf16)
        nc.sync.dma_start(wm3_sb[:], wm3[:])
        eidx_sb = cst.tile([128, EPADP // 16], i16)
        nc.sync.dma_start(eidx_sb[:], eidx[:])
        rloc_sb = cst.tile([128, NCHP], f32)
        nc.sync.dma_start(rloc_sb[:], rlocT[:])
        aT_sb = cst.tile([128, NCHP * 3], f32)
        nc.sync.dma_start(aT_sb[:], aT[:])
        q_sb = cst.tile([128, NCHP * 8], f32)

        # ---- stages A (sc), C (MLP+Q), then premp-all; one PSUM scope ------
        with tc.tile_pool(name="sa", bufs=1) as sa, \
             tc.tile_pool(name="sa2", bufs=3) as sa2, \
             tc.tile_pool(name="mc", bufs=3) as mc, \
             tc.tile_pool(name="psA", bufs=2, space="PSUM") as psA:
            wsc0_sb = sa.tile([128, S * 128], f32)
            nc.sync.dma_start(wsc0_sb[:], wsc0T[:])
            wsc1_sb = sa.tile([128, S * 128], f32)
            nc.sync.dma_start(wsc1_sb[:], wsc1T[:])
            ef_sb = sa.tile([8, EPADP], bf16)
            nc.sync.dma_start(ef_sb[:], efT[:])
            wpreb_sb = sa.tile([128, 256], bf16)
            nc.sync.dma_start(wpreb_sb[:], wpre_b[:])
            nfc = []
            for comp in range(4):
                t = sa.tile([128, NSLOT], f32, name=f"nfc{comp}")
                nc.gpsimd.dma_start(t[:], nfT[comp, :, :])
                nfc.append(t)
            # ---- A: per-species sc for the local shard ----
            for gt in range(NGT):
                sp = species_of_tile[gt]
                ps_sc = psA.tile([128, 512], f32, tag="ps_sc", space="PSUM")
                mm_chain = []
                for comp in range(4):
                    lhsT = nfc[comp][:, gt * 128:(gt + 1) * 128]
                    wsc = (wsc0_sb if comp == 0 else wsc1_sb)[:, sp * 128:(sp + 1) * 128]
                    mm_chain.append(nc.tensor.matmul(
                        ps_sc[:, comp * 128:(comp + 1) * 128],
                        lhsT=lhsT, rhs=wsc, start=True, stop=True))
                for a, b in zip(mm_chain, mm_chain[1:]):
                    _dep(b, a)
                sc_sb = sa2.tile([128, 512], f32, tag="sc_sb")
                nc.vector.tensor_copy(sc_sb[:], ps_sc[:])
                nc.gpsimd.dma_start(sc_out[gt * 128:(gt + 1) * 128, :], sc_sb[:])
            # ---- C: radial MLP + per-edge scalar table Q ----
            for gm in range(NCHP // 32):   # 32 chunks = 4096 edges per group
                ps_mix = psA.tile([128, 128], f32, tag="ps_mix", space="PSUM")
                mix_chain = []
                for g5 in range(8):        # 512-edge subgroups
                    e0 = gm * 4096 + g5 * 512
                    ps_h = psA.tile([64, 512], f32, tag="ps_h", space="PSUM")
                    nc.tensor.matmul(ps_h[:], lhsT=wm1_sb[:], rhs=ef_sb[:, e0:e0 + 512],
                                     start=True, stop=True)
                    h1 = mc.tile([64, 512], bf16, tag="h1")
                    nc.scalar.activation(h1[:], ps_h[:], mybir.ActivationFunctionType.Silu)
                    ps_h2 = psA.tile([64, 512], f32, tag="ps_h2", space="PSUM")
                    nc.tensor.matmul(ps_h2[:], lhsT=wm2_sb[:], rhs=h1[:],
                                     start=True, stop=True)
                    h2 = mc.tile([64, 512], bf16, tag="h2")
                    nc.scalar.activation(h2[:], ps_h2[:], mybir.ActivationFunctionType.Silu)
                    for j in range(4):
                        cc = g5 * 4 + j
                        mix_chain.append(nc.tensor.matmul(
                            ps_mix[:, cc * 4:cc * 4 + 4],
                            lhsT=h2[:, j * 128:(j + 1) * 128], rhs=wm3_sb[:],
                            start=True, stop=True))
                for a, b in zip(mix_chain, mix_chain[1:]):
                    _dep(b, a)
                mix_sb = mc.tile([128, 128], f32, tag="mix_sb")
                nc.vector.tensor_copy(mix_sb[:], ps_mix[:])
                # Q build for these 32 chunks
                qs = q_sb[:, gm * 256:(gm + 1) * 256]
                mix4 = mix_sb[:].rearrange("p (c f) -> p c f", f=4)
                q8 = qs.rearrange("p (c f) -> p c f", f=8)
                a3 = aT_sb[:, gm * 96:(gm + 1) * 96].rearrange("p (c f) -> p c f", f=3)
                nc.vector.tensor_copy(q8[:, :, 0:1], mix4[:, :, 0:1])
                nc.vector.tensor_tensor(out=q8[:, :, 1:4], in0=a3[:, :, :],
                                        in1=mix4[:, :, 1:2].to_broadcast([128, 32, 3]),
                                        op=mybir.AluOpType.mult)
                nc.vector.tensor_copy(q8[:, :, 4:5], mix4[:, :, 2:3])
                nc.vector.tensor_tensor(out=q8[:, :, 5:8], in0=a3[:, :, :],
                                        in1=mix4[:, :, 3:4].to_broadcast([128, 32, 3]),
                                        op=mybir.AluOpType.mult)
            # ---- premp for ALL cores' nodes (replaces the AllGather) ------
            NR = N
            for g8 in range(NR // 1024):
                nfb = []
                for comp in range(4):
                    tb = sa2.tile([128, 1024], bf16, tag=f"nfb{comp}", name=f"nfb{comp}")
                    nc.gpsimd.dma_start(tb[:], nfTb[comp, :, g8 * 1024:(g8 + 1) * 1024])
                    nfb.append(tb)
                f_wide = sa2.tile([128, 8, 512], bf16, tag="f_wide")
                for sub in range(8):
                    ps_f = psA.tile([128, 512], f32, tag="ps_sc", space="PSUM")
                    fchain = []
                    for comp in range(4):
                        wpr = wpreb_sb[:, 0:128] if comp == 0 else wpreb_sb[:, 128:256]
                        fchain.append(nc.tensor.matmul(
                            ps_f[:, comp * 128:(comp + 1) * 128],
                            lhsT=nfb[comp][:, sub * 128:(sub + 1) * 128],
                            rhs=wpr, start=True, stop=True))
                    for a, b in zip(fchain, fchain[1:]):
                        _dep(b, a)
                    nc.vector.tensor_copy(f_wide[:, sub, :], ps_f[:])
                nc.sync.dma_start(
                    F_full[g8 * 1024:(g8 + 1) * 1024, :].rearrange(
                        "(s p) c -> p s c", p=128),
                    f_wide[:])

        # ---------------- stage D: gather / scatter / postmp ----------------
        with tc.tile_pool(name="sd", bufs=3) as sd, \
             tc.tile_pool(name="sdh", bufs=4) as sdh, \
             tc.tile_pool(name="psagg", bufs=2, space="PSUM") as psagg, \
             tc.tile_pool(name="pso", bufs=2, space="PSUM") as pso:
            for t in range(NT):
                nch = chunks_t[t]
                c0 = off_t[t]
                fg = sd.tile([128, MAXCH, 512], bf16, tag="fg")
                oh_sl = sd.tile([128, MAXCH, 128], bf16, tag="oh_sl")
                nc.sync.dma_start(oh_sl[:, :nch, :],
                                  ohT[:, c0 * 128:(c0 + nch) * 128].rearrange(
                                      "p (j n) -> p j n", n=128))
                # dma_gather ucode handles at most 1024 indices per call
                for b0 in range(0, nch, 8):
                    bn = min(8, nch - b0)
                    nc.gpsimd.dma_gather(
                        fg[:, b0:b0 + bn, :], F_full[:],
                        eidx_sb[:, (c0 + b0) * 8:(c0 + b0 + bn) * 8],
                        bn * 128, bn * 128, 512)
                agg = psagg.tile([128, 1024], f32, tag="agg", space="PSUM")
                # PSUM accumulation groups are per 2KB bank: exactly one MM
                # per bank carries start=True (pending-zeroes the whole bank),
                # one carries stop=True, and explicit deps order them.
                bank_mms = [[], []]  # instructions per psum bank
                for j in range(nch):
                    ch = c0 + j
                    rl = rloc_sb[:, ch:ch + 1]
                    q = [q_sb[:, ch * 8 + i:ch * 8 + i + 1] for i in range(8)]
                    h = [sdh.tile([128, 128], bf16, tag=f"h{i}", name=f"h{i}")
                         for i in range(8)]
                    # DVE: fused (iota==rloc)*q at 4x mode; ACT: scaled
                    # copies of the host-precomputed one-hot slab.  Alternate
                    # 7/1 and 6/2 splits to balance the two engines.
                    ndve = 7 if j % 2 == 0 else 6
                    for i in range(ndve):
                        nc.vector.tensor_scalar(
                            h[i][:], iota_sb[:], rl, q[i][:],
                            mybir.AluOpType.is_equal, mybir.AluOpType.mult)
                    for i in range(ndve, 8):
                        nc.scalar.mul(h[i][:], oh_sl[:, j, :], q[i][:])
                    h = [t_[:] for t_ in h]
                    fs = fg[:, j, 0:128]
                    fvx = fg[:, j, 128:256]
                    fvy = fg[:, j, 256:384]
                    fvz = fg[:, j, 384:512]
                    # (column block, lhsT, rhs): block 0: s*mix0; 1: dot;
                    # 2-4: v_i*mix2; 5-7: s*a_i*mix3
                    for col, lhsT, rhs in (
                        (0, fs, h[0]), (640, fs, h[5]), (768, fs, h[6]),
                        (896, fs, h[7]), (128, fvx, h[1]), (256, fvx, h[4]),
                        (128, fvy, h[2]), (384, fvy, h[4]), (128, fvz, h[3]),
                        (512, fvz, h[4]),
                    ):
                        bank = col // 512
                        first = j == 0 and not bank_mms[bank]
                        inst = nc.tensor.matmul(
                            agg[:, col:col + 128], lhsT=lhsT, rhs=rhs,
                            start=first, stop=False)
                        bank_mms[bank].append(inst)
                for bank in range(2):
                    mms = bank_mms[bank]
                    mms[-1].ins.stop_tensor_calc = True
                    for m in mms[1:]:
                        _dep(m, mms[0])
                    for m in mms[:-1]:
                        _dep(mms[-1], m)
                agg_sb = sd.tile([128, 1024], bf16, tag="agg_sb")
                nc.scalar.copy(agg_sb[:], agg[:])
                o_ps = pso.tile([128, 512], f32, tag="o_ps", space="PSUM")
                # os = aggT0 @ Wp00 + aggT1 @ Wp01 ; ov_i = aggT(2+i) @ Wp10 + aggT(5+i) @ Wp11
                o_chain = [nc.tensor.matmul(
                    o_ps[:, 0:128], lhsT=agg_sb[:, 0:128], rhs=wp_sb[:, 0:128],
                    start=True, stop=False)]
                o_chain.append(nc.tensor.matmul(
                    o_ps[:, 0:128], lhsT=agg_sb[:, 128:256], rhs=wp_sb[:, 128:256],
                    start=False, stop=True))
                for i in range(3):
                    o_chain.append(nc.tensor.matmul(
                        o_ps[:, 128 * (i + 1):128 * (i + 2)],
                        lhsT=agg_sb[:, 128 * (2 + i):128 * (3 + i)],
                        rhs=wp_sb[:, 256:384], start=True, stop=False))
                    o_chain.append(nc.tensor.matmul(
                        o_ps[:, 128 * (i + 1):128 * (i + 2)],
                        lhsT=agg_sb[:, 128 * (5 + i):128 * (6 + i)],
                        rhs=wp_sb[:, 384:512], start=False, stop=True))
                for a, b in zip(o_chain, o_chain[1:]):
                    _dep(b, a)
                out_sb = sd.tile([128, 512], f32, tag="out_sb")
                nc.scalar.copy(
                    out_sb[:].rearrange("p (d c) -> p c d", c=4),
                    o_ps[:].rearrange("p (c d) -> p c d", c=4))
                nc.sync.dma_start(nout[t * 128:(t + 1) * 128, :], out_sb[:])

    nc.compile()
    return nc


def kernel(**inputs):
    meta, per_core, shared, unshard = _host_prep(**inputs)
    key = (meta["NSLOT"], meta["NCH"], meta["chunks_t"], meta["species_of_tile"])
    if key not in _CACHE:
        _CACHE[key] = _build(meta)
    nc = _CACHE[key]
    in_maps = [dict(pc, **shared) for pc in per_core]
    res = run_bass_kernel_spmd(nc, in_maps, core_ids=list(range(NCORES)))
    node_out = np.concatenate([res.results[k]["nout"] for k in range(NCORES)], axis=0)
    node_out = node_out.reshape(N, 128, 4)
    node_of = unshard["node_of"]
    sc = np.zeros((N, 128, 4), np.float32)
    for k in range(NCORES):
        valid = node_of[k] >= 0
        rows = res.results[k]["sc_out"][valid]
        sc[k * NSH + node_of[k][valid]] = rows.reshape(-1, 4, 128).transpose(0, 2, 1)
    return node_out, sc
